# revision 21
# baseline (speedup 1.0000x reference)
"""Multi-head attention (B=4, S=2048, H=8, Dh=64, Dm=512) on 8 TRN2 NeuronCores.

Sharding: batch*head parallel. Core c owns batch b = c//2 and head group
g = c%2 (4 heads each). Each core computes QKV projection for its head
group, transposed-scores flash-style attention (no max subtraction --
scores ~ N(0,1) after 1/sqrt(Dh) scaling, exp is safe in fp32/bf16), and
its partial output projection against its 256 rows of Wo. The host sums
the two partial projections per batch.

Device-side layout notes:
  - X^T (bf16) is prepared on host so every matmul contracts over the
    partition dim directly.
  - Scores are computed transposed (S^T[j,i] = K Q^T) so the attention*V
    matmul needs no transposition; the two heads of a 128-row Q^T/K^T
    chunk are packed into the PE array as two K=64 row-tiles (tile_position
    (0,0)/(64,0)) running concurrently.
  - Row sums of exp(scores) come for free from a ones-column appended to V
    (M=65 stationary); normalization uses an fp16 K=1 broadcast matmul +
    DVE fast-reciprocal/multiply, emitted lazily into the next block so the
    in-order PE stream never stalls at block boundaries.
  - Schedule: exp(scores) on ScalarE is the critical engine; the lead
    emits only the 3 Q/K chunks block 0 strictly needs, and all other
    projections/normalization interleave into the attention j-loops at
    one-matmul granularity to keep both PE and ACT dense.
"""

import os
import sys

for _p in ("/opt/trn_rl_repo",):
    if os.path.isdir(_p) and _p not in sys.path:
        sys.path.append(_p)

import ml_dtypes
import numpy as np

import concourse.bass as bass
import concourse.tile as tile
from concourse import bacc, mybir
from concourse.bass_utils import run_bass_kernel_spmd

BF16 = mybir.dt.bfloat16
F16 = mybir.dt.float16
F32 = mybir.dt.float32

B, S, DM = 4, 2048, 512
H, DH = 8, 64
HPC = 4  # heads per core
DQ = HPC * DH  # 256: per-core slice of the inner dim
N_CORES = 8
SCALE = DH**-0.5

AF = mybir.ActivationFunctionType

# exported for test harnesses
LAST_EXEC_TIME_NS = None
LAST_RESULT = None

_CACHED_NC = None


def _kernel_body(tc, xT_d, wq_d, wk_d, wv_d, wo_d, out_d):
    from contextlib import ExitStack

    nc = tc.nc
    with ExitStack() as ctx:
        consts = ctx.enter_context(tc.tile_pool(name="consts", bufs=1))
        ptp = ctx.enter_context(tc.tile_pool(name="pt", bufs=10))
        normp = ctx.enter_context(tc.tile_pool(name="norm", bufs=3))
        foutp = ctx.enter_context(tc.tile_pool(name="fout", bufs=4))
        # PSUM budget (8 banks): "s" 2x[128,1024]=4, "o" 3x[65,512]=3, "x" 1
        ps_s = ctx.enter_context(tc.tile_pool(name="ps_s", bufs=2, space="PSUM"))
        ps_o = ctx.enter_context(tc.tile_pool(name="ps_o", bufs=3, space="PSUM"))
        ps_x = ctx.enter_context(tc.tile_pool(name="ps_x", bufs=1, space="PSUM"))

        sb_xT = consts.tile([128, 4, S], BF16)  # X^T: k-chunk c -> [:, c, :]
        sb_wq = consts.tile([128, 4, DQ], BF16)
        sb_wk = consts.tile([128, 4, DQ], BF16)
        sb_wv = consts.tile([128, 4, DQ], BF16)
        sb_wo = consts.tile([128, 2, DM], BF16)  # d'-chunk p -> [:, p, :]
        sb_qT = consts.tile([128, 2, S], BF16)  # dq-chunk (head pair) p
        sb_kT = consts.tile([128, 2, S], BF16)
        sb_v = consts.tile([128, 16, HPC, 66], BF16)  # V_aug; col 64 = ones
        sb_oT = consts.tile([128, 2, S], BF16)  # normalized O^T
        sb_warm = consts.tile([128, 512], BF16)  # PE warmup fodder
        sb_one = consts.tile([128, 64], F16)  # all-ones (bcast stationary)
        # bottom half of Wo's pair-1 rows re-homed at partitions 0:64 so the
        # last block's head-1 output never needs a cross-partition DMA
        sb_wo2 = consts.tile([64, DM], BF16)
        sb_stage = consts.tile([64, 512], BF16)  # last-block head-1 oT

        # sb_warm via GPSIMD: that queue inits ~1.5us before DVE, so the PE
        # warmup (gated only on this memset) starts correspondingly earlier
        nc.gpsimd.memset(sb_warm[:], 1.0)
        nc.vector.memset(sb_one[:], 1.0)
        nc.vector.memset(sb_v[:, :, :, 64:66], 1.0)
        # Input DMAs: all on ONE queue (splitting across queues just splits
        # the shared ~356GB/s HBM bandwidth and slows the critical prefix).
        # Strict need-order with 512-column X^T slices so the lead matmuls
        # pipeline with the arriving data: Q0c0 is unblocked after ~0.8MB
        # instead of the full 2.8MB.
        xT_r = xT_d.rearrange("(c p) s -> c p s", p=128)
        nc.sync.dma_start(sb_wq[:], wq_d.rearrange("(c p) d -> p c d", p=128))
        for kc in range(4):
            nc.sync.dma_start(sb_xT[:, kc, 0:512], xT_r[kc][:, 0:512])
        nc.sync.dma_start(sb_wk[:], wk_d.rearrange("(c p) d -> p c d", p=128))
        nc.sync.dma_start(sb_wv[:], wv_d.rearrange("(c p) d -> p c d", p=128))
        for kc in range(4):
            nc.sync.dma_start(sb_xT[:, kc, 512:1024], xT_r[kc][:, 512:1024])
        for kc in range(4):
            nc.sync.dma_start(sb_xT[:, kc, 1024:1536], xT_r[kc][:, 1024:1536])
        for kc in range(4):
            nc.sync.dma_start(sb_xT[:, kc, 1536:2048], xT_r[kc][:, 1536:2048])
        nc.sync.dma_start(sb_wo[:], wo_d.rearrange("(c p) d -> p c d", p=128))
        nc.sync.dma_start(sb_wo2[:], wo_d[192:256, :])

        # Warm the PE (HAM un-throttle needs ~3.4us of sustained matmul) and
        # preload the exp table while the first DMAs are in flight; the lead
        # matmuls themselves continue the warmup as data lands.
        pw = ps_x.tile([128, 512], F32, tag="x")
        for r in range(9):
            nc.tensor.matmul(
                pw[:], lhsT=sb_warm[:, 0:128], rhs=sb_warm[:], start=True, stop=True
            )
        warm_act = normp.tile([1, 4], F32, tag="wact")
        nc.scalar.activation(warm_act[:], pw[0:1, 0:4], AF.Exp, scale=-1.0)

        def emit_qk_chunk(w_sb, dst_sb, p, c, pool_tag=("ps_s", "s")):
            """One [128,512] chunk of Q^T or K^T for head-pair p."""
            isl = slice(c * 512, (c + 1) * 512)
            pool = {"ps_s": ps_s, "ps_o": ps_o, "ps_x": ps_x}[pool_tag[0]]
            pq = pool.tile([128, 512], F32, tag=pool_tag[1], name="pqk")
            for kc in range(4):
                nc.tensor.matmul(
                    pq[:],
                    lhsT=w_sb[:, kc, p * 128 : (p + 1) * 128],
                    rhs=sb_xT[:, kc, isl],
                    start=(kc == 0),
                    stop=(kc == 3),
                )
            nc.vector.tensor_copy(dst_sb[:, p, isl], pq[:])

        def emit_qk_chunk_mm(w_sb, p, c, kc, pq):
            nc.tensor.matmul(
                pq[:],
                lhsT=w_sb[:, kc, p * 128 : (p + 1) * 128],
                rhs=sb_xT[:, kc, c * 512 : (c + 1) * 512],
                start=(kc == 0),
                stop=(kc == 3),
            )

        def emit_v_chunk(sc):
            """V natural [s,dv] for s-chunk sc (all 4 heads)."""
            pv = ps_x.tile([128, DQ], F32, tag="x", name="pv")
            for kc in range(4):
                nc.tensor.matmul(
                    pv[:],
                    lhsT=sb_xT[:, kc, sc * 128 : (sc + 1) * 128],
                    rhs=sb_wv[:, kc, :],
                    start=(kc == 0),
                    stop=(kc == 3),
                )
            nc.vector.tensor_copy(
                sb_v[:, sc, :, 0:64], pv.rearrange("p (h d) -> p h d", h=HPC)
            )

        # ---- lead: exactly what block 0 strictly needs, in DMA-arrival
        # order: Q^T chunks 0,1 and K^T chunk 0 for pair 0, plus V chunks
        # 0-3 (which fill the PE-idle window while the second X^T S-quarter
        # is still landing). Everything else streams into the j-loops. ----
        emit_qk_chunk(sb_wq, sb_qT, 0, 0, ("ps_o", "o"))
        emit_qk_chunk(sb_wk, sb_kT, 0, 0)
        for sc in range(4):
            emit_v_chunk(sc)
        emit_qk_chunk(sb_wq, sb_qT, 0, 1, ("ps_o", "o"))

        # deferred QK work, flattened to per-MM granularity: remaining pair-0
        # Q chunks, then all pair-1 K and Q chunks
        pending_qk = [(sb_wq, sb_qT, 0, 2), (sb_wq, sb_qT, 0, 3)]
        for c in range(4):
            pending_qk.append((sb_wk, sb_kT, 1, c))
        for c in range(4):
            pending_qk.append((sb_wq, sb_qT, 1, c))
        qk_state = {"chunk": None, "tile": None, "kc": 0}

        def step_pending_qk():
            stt = qk_state
            if stt["chunk"] is None:
                if not pending_qk:
                    return
                stt["chunk"] = pending_qk.pop(0)
                stt["tile"] = ps_x.tile([128, 512], F32, tag="x", name="pqk1")
                stt["kc"] = 0
            w_sb, dst_sb, p, c = stt["chunk"]
            emit_qk_chunk_mm(w_sb, p, c, stt["kc"], stt["tile"])
            stt["kc"] += 1
            if stt["kc"] == 4:
                nc.vector.tensor_copy(
                    dst_sb[:, p, c * 512 : (c + 1) * 512], stt["tile"][:]
                )
                stt["chunk"] = None

        # ---- attention: pair 0 then pair 1 ----
        # Normalization of block k is emitted lazily, interleaved into the
        # first iterations of block k+1, so the in-order PE stream never
        # stalls long enough for HAM to re-throttle the clock.
        def make_norm_steps(p, ic, po, last=False):
            """Normalization of a finished block, split into 3 steps that the
            next block interleaves into its first iterations (the fp16 K=1
            broadcast matmuls sit behind fresh scores in PE order, so the PE
            never stalls waiting on the DVE sums copies). For the last block,
            head 1's result goes to the partitions-0:63 staging tile (read
            against sb_wo2) instead of the slow cross-partition DMA."""
            isl = slice(ic * 512, (ic + 1) * 512)
            held = {}

            def step_sums():
                for hi in range(2):
                    s = normp.tile([65, 512], F16, tag="sums", name=f"sums{hi}")
                    nc.vector.tensor_copy(s[64:65, :], po[hi][64:65, :])
                    held[hi] = s

            def step_head(hi):
                if last and hi == 1:
                    # tail: the spare "o" slot is free; avoids serializing
                    # behind head 0's pb in the single "x" slot
                    pb = ps_o.tile([64, 512], F32, tag="o", name="pbz")
                else:
                    pb = ps_x.tile([64, 512], F32, tag="x", name=f"pb{hi}")
                nc.tensor.matmul(
                    pb[:],
                    lhsT=sb_one[64:65, :],
                    rhs=held[hi][64:65, :],
                    start=True,
                    stop=True,
                )
                rec = normp.tile([64, 512], F32, tag="rec", name=f"rec{hi}")
                nc.vector.reciprocal_approx_fast(rec[:], pb[:])
                if hi == 0:
                    nc.vector.tensor_mul(
                        sb_oT[0:64, p, isl], po[0][0:64, :], rec[:]
                    )
                elif last:
                    nc.vector.tensor_mul(sb_stage[:], po[1][0:64, :], rec[:])
                else:
                    tmpb = normp.tile([64, 512], BF16, tag="tmpb")
                    nc.vector.tensor_mul(tmpb[:], po[1][0:64, :], rec[:])
                    nc.sync.dma_start(sb_oT[64:128, p, isl], tmpb[:])

            return [step_sums, lambda: step_head(0), lambda: step_head(1)]

        # per-MM-granularity deferred projection chunks (run during p1 blocks)
        pending_proj = []
        proj_state = {"c2": None, "tile": None, "p": 0}

        def step_pending_proj():
            stt = proj_state
            if stt["c2"] is None:
                if not pending_proj:
                    return
                stt["c2"] = pending_proj.pop(0)
                stt["tile"] = ps_x.tile([128, 512], F32, tag="x", name="pf")
                stt["p"] = 0
            c2, p = stt["c2"], stt["p"]
            nc.tensor.matmul(
                stt["tile"][:],
                lhsT=sb_oT[:, p, c2 * 128 : (c2 + 1) * 128],
                rhs=sb_wo[:, p, :],
                start=(p == 0),
                stop=(p == 1),
            )
            stt["p"] += 1
            if stt["p"] == 2:
                fo = foutp.tile([128, 512], F32, tag="fo")
                nc.vector.tensor_copy(fo[:], stt["tile"][:])
                nc.sync.dma_start(out_d[c2 * 128 : (c2 + 1) * 128, :], fo[:])
                stt["c2"] = None

        def step_extras():
            """One deferred matmul: pending QK first, then projections."""
            if pending_qk or qk_state["chunk"] is not None:
                step_pending_qk()
            else:
                step_pending_proj()

        pending_norm = []
        blocks = [(p, ic) for p in range(2) for ic in range(4)]

        def emit_scores(p, ic, j):
            isl = slice(ic * 512, (ic + 1) * 512)
            jsl = slice(j * 128, (j + 1) * 128)
            st = ps_s.tile([128, 1024], F32, tag="s")
            nc.tensor.matmul(
                st[:, 0:512],
                lhsT=sb_kT[0:64, p, jsl],
                rhs=sb_qT[0:64, p, isl],
                start=True,
                stop=True,
            )
            nc.tensor.matmul(
                st[:, 512:1024],
                lhsT=sb_kT[64:128, p, jsl],
                rhs=sb_qT[64:128, p, isl],
                start=True,
                stop=True,
            )
            return st

        def emit_exp(st):
            pt = ptp.tile([128, 1024], BF16, tag="pt")
            nc.scalar.activation(pt[:], st[:], AF.Exp, scale=SCALE)
            return pt

        # Global score/exp stream runs TWO iterations ahead of the AV
        # stream: scores(k+2) issue before iteration k's AVs, so at block
        # boundaries the next block's first exps never queue behind the
        # last AVs (which themselves wait on the final exp of the block).
        # The 2-slot score pool self-paces the stream: scores(k+2) can't
        # run before EXP(k) has drained its slot.
        sched = [(p, ic, j) for (p, ic) in blocks for j in range(16)]
        pts = {}

        def emit_scores_exp(k):
            p, ic, j = sched[k]
            pts[k] = emit_exp(emit_scores(p, ic, j))

        emit_scores_exp(0)
        emit_scores_exp(1)
        po = None
        pending_burst = None

        def make_av(p_, po_):
            def emit_av(hi, jj, ptt):
                nc.tensor.matmul(
                    po_[hi][:],
                    lhsT=sb_v[:, jj, 2 * p_ + hi, 0:65],
                    rhs=ptt[:, hi * 512 : (hi + 1) * 512],
                    start=(jj == 0),
                    stop=(jj == 15),
                    skip_group_check=True,
                )

            return emit_av

        for k, (p, ic, j) in enumerate(sched):
            if j == 0:
                po = [
                    ps_o.tile([65, 512], F32, tag="o", name=f"po{hi}")
                    for hi in range(2)
                ]
                emit_av = make_av(p, po)
                if p == 1 and ic > 0:
                    # previous ic's projection slice; its oT inputs complete
                    # during this block's first two iterations (lazy norm)
                    pending_proj.extend(range(4 * (ic - 1), 4 * ic))
            if k + 2 < len(sched):
                emit_scores_exp(k + 2)
            if pending_burst is not None:
                # previous block's last three AVs + sums, emitted AFTER this
                # iteration's score pair so the next exps never queue behind
                # AVs that themselves wait on the previous block's last exp
                pending_burst()
                pending_burst = None
            if pending_norm:
                if j == 1:
                    pending_norm[0]()  # bcast+recip+mul head 0
                elif j == 2:
                    pending_norm[1]()  # ... head 1
                    pending_norm = []
            # extras: deferred matmuls keep PE fed; x-slot is needed
            # by the norm broadcasts at j=1,2 so extras wait till j>=3
            # (block 1 starts later still: it carries the densest deferred
            # stream and was PE-oversubscribed at j=3,4,5)
            if p == 0 and ic == 0:
                # K chunks 1-3 at 2 matmuls/iter over j=0..5; chunk c's copy
                # lands at j=2c-1, one iter before the 2-ahead score stream
                # for j=4c reads it
                if j <= 5:
                    c, half = 1 + j // 2, j % 2
                    if half == 0:
                        kq_tile = ps_o.tile([128, 512], F32, tag="o", name="pkh")
                    for kc in (0, 1) if half == 0 else (2, 3):
                        emit_qk_chunk_mm(sb_wk, 0, c, kc, kq_tile)
                    if half == 1:
                        nc.vector.tensor_copy(
                            sb_kT[:, 0, c * 512 : (c + 1) * 512], kq_tile[:]
                        )
                if 3 <= j < 15:
                    emit_v_chunk(j + 1)
            elif j >= (6 if (p == 0 and ic == 1) else 5):
                step_extras()

            # uniform AV lag (h0 by 1 iter, h1 by 2) keeps scores ahead
            # of the AV stream so ACT never waits at block boundaries
            if j >= 1:
                emit_av(0, j - 1, pts[k - 1])
            if j >= 2:
                emit_av(1, j - 2, pts[k - 2])
            if j == 15:

                def mk_burst(av_, p_, ic_, po_, k_, last_):
                    def burst():
                        nonlocal pending_norm
                        av_(0, 15, pts[k_])
                        av_(1, 14, pts[k_ - 1])
                        av_(1, 15, pts[k_])
                        pending_norm = make_norm_steps(p_, ic_, po_, last=last_)
                        pending_norm[0]()  # sums copies right behind the AVs
                        pending_norm = pending_norm[1:]

                    return burst

                pending_burst = mk_burst(
                    emit_av, p, ic, po, k, k == len(sched) - 1
                )
        pending_burst()  # last block's AV tail + sums

        # ---- tail: last normalize + remaining projection chunks ----
        # head 0 first: its chain is pure DVE and unblocks the oT top half
        pending_norm[0]()
        pending_norm[1]()
        while pending_proj or proj_state["c2"] is not None:
            step_pending_proj()
        # final four chunks, packed two per score-pool tile (those slots are
        # free the moment the last EXP consumed them): the pair-0 partials
        # depend only on long-finished oT pair-0 rows, so all four issue
        # immediately and overlap the norm chain above. The pair-1 matmuls
        # are split K=64: the top halves run off head 0's DVE-written oT
        # rows; the bottom halves contract the partitions-0:63 staging tile
        # against sb_wo2, so no cross-partition DMA gates the tail.
        # Groups interleave across tiles (skip_group_check).
        pfz = [ps_s.tile([128, 1024], F32, tag="s", name=f"pfz{n}") for n in range(2)]
        slots = [(c2, pfz[n // 2][:, (n % 2) * 512 : (n % 2 + 1) * 512])
                 for n, c2 in enumerate(range(12, 16))]
        for c2, pf in slots:
            nc.tensor.matmul(
                pf,
                lhsT=sb_oT[:, 0, c2 * 128 : (c2 + 1) * 128],
                rhs=sb_wo[:, 0, :],
                start=True,
                stop=False,
                skip_group_check=True,
            )
        for c2, pf in slots:
            nc.tensor.matmul(
                pf,
                lhsT=sb_oT[0:64, 1, c2 * 128 : (c2 + 1) * 128],
                rhs=sb_wo[0:64, 1, :],
                start=False,
                stop=False,
                skip_group_check=True,
            )
        for c2, pf in slots:
            nc.tensor.matmul(
                pf,
                lhsT=sb_stage[:, (c2 - 12) * 128 : (c2 - 11) * 128],
                rhs=sb_wo2[:],
                start=False,
                stop=True,
                skip_group_check=True,
            )
        for c2, pf in slots:
            fo = foutp.tile([128, 512], F32, tag="fo")
            nc.vector.tensor_copy(fo[:], pf)
            nc.sync.dma_start(out_d[c2 * 128 : (c2 + 1) * 128, :], fo[:])


def _build():
    nc = bacc.Bacc("TRN2", target_bir_lowering=False, debug=False, num_devices=N_CORES)
    xT = nc.dram_tensor("xT", [DM, S], BF16, kind="ExternalInput")
    wq = nc.dram_tensor("wq", [DM, DQ], BF16, kind="ExternalInput")
    wk = nc.dram_tensor("wk", [DM, DQ], BF16, kind="ExternalInput")
    wv = nc.dram_tensor("wv", [DM, DQ], BF16, kind="ExternalInput")
    wo = nc.dram_tensor("wo", [DQ, DM], BF16, kind="ExternalInput")
    out = nc.dram_tensor("out", [S, DM], F32, kind="ExternalOutput")
    with tile.TileContext(nc) as tc:
        _kernel_body(tc, xT.ap(), wq.ap(), wk.ap(), wv.ap(), wo.ap(), out.ap())
    nc.compile()
    return nc


def get_nc():
    global _CACHED_NC
    if _CACHED_NC is None:
        _CACHED_NC = _build()
    return _CACHED_NC


def _in_maps(hidden_states, Wq, Wk, Wv, Wo):
    bf = ml_dtypes.bfloat16
    maps = []
    for c in range(N_CORES):
        b, g = c // 2, c % 2
        cols = slice(g * DQ, (g + 1) * DQ)
        maps.append(
            {
                "xT": np.ascontiguousarray(hidden_states[b].T).astype(bf),
                "wq": np.ascontiguousarray(Wq[:, cols]).astype(bf),
                "wk": np.ascontiguousarray(Wk[:, cols]).astype(bf),
                "wv": np.ascontiguousarray(Wv[:, cols]).astype(bf),
                "wo": np.ascontiguousarray(Wo[cols, :]).astype(bf),
            }
        )
    return maps


def _ensure_profile_support():
    """Best-effort: register the axon NTFF profiling hook + defang the
    bucket upload (zero-egress container). Without this, trace=True dies
    on a missing ``antenv.axon_hooks`` module in this image."""
    import types

    try:
        import antenv

        if "antenv.axon_hooks" not in sys.modules:
            mod = types.ModuleType("antenv.axon_hooks")
            _h = {"hook": None}
            mod.set_axon_ntff_profile_hook = lambda h: _h.__setitem__("hook", h)
            mod.get_axon_ntff_profile_hook = lambda: _h["hook"]
            sys.modules["antenv.axon_hooks"] = mod
            antenv.axon_hooks = mod
        import antenv.axon_hooks as ah

        if ah.get_axon_ntff_profile_hook() is None:
            if "/root/.axon_site" not in sys.path:
                sys.path.append("/root/.axon_site")
            from trn_agent_boot.trn_boot import _ntff_profile_via_ctypes

            hook = _ntff_profile_via_ctypes("/opt/axon/libaxon_pjrt.so")
            if hook is not None:
                ah.set_axon_ntff_profile_hook(hook)
    except Exception:
        pass
    try:
        import concourse.bass_utils as bu

        bu.upload_artifacts = lambda tmpdir: tmpdir
    except Exception:
        pass


def kernel(hidden_states, Wq, Wk, Wv, Wo):
    global LAST_EXEC_TIME_NS, LAST_RESULT
    hidden_states = np.asarray(hidden_states, dtype=np.float32)
    Wq, Wk, Wv, Wo = (np.asarray(w, dtype=np.float32) for w in (Wq, Wk, Wv, Wo))

    trace = bool(os.environ.get("BASS_TRACE"))
    if trace:
        _ensure_profile_support()
    nc = get_nc()
    maps = _in_maps(hidden_states, Wq, Wk, Wv, Wo)
    res = run_bass_kernel_spmd(
        nc,
        maps,
        core_ids=list(range(N_CORES)),
        trace=trace,
        tmpdir=os.environ.get("BASS_TRACE_DIR") or None,
    )
    LAST_RESULT = res
    LAST_EXEC_TIME_NS = res.exec_time_ns

    out = np.empty((B, S, DM), dtype=np.float32)
    for b in range(B):
        out[b] = res.results[2 * b]["out"] + res.results[2 * b + 1]["out"]
    return out


if __name__ == "__main__":
    rng = np.random.default_rng(0)
    hs = rng.standard_normal((B, S, DM), dtype=np.float32)
    ws = [
        (rng.standard_normal((DM, DM), dtype=np.float32) / np.sqrt(DM))
        for _ in range(4)
    ]
    o = kernel(hs, *ws)
    print("out", o.shape, o.dtype, float(np.abs(o).mean()))
    print("exec_time_ns", LAST_EXEC_TIME_NS)


# revision 25
# speedup vs baseline: 1.1706x; 1.1706x over previous
"""Multi-head attention (B=4, S=2048, H=8, Dh=64, Dm=512) on 8 TRN2 NeuronCores.

Sharding: batch*head parallel. Core c owns batch b = c//2 and head group
g = c%2 (4 heads each). Each core computes QKV projection for its head
group, transposed-scores flash-style attention (no max subtraction --
scores ~ N(0,1) after 1/sqrt(Dh) scaling, exp is safe in fp32/bf16), and
its partial output projection against its 256 rows of Wo. The host sums
the two partial projections per batch.

Device-side layout notes:
  - X^T (bf16) is prepared on host so every matmul contracts over the
    partition dim directly.
  - Scores are computed transposed (S^T[j,i] = K Q^T) so the attention*V
    matmul needs no transposition; the two heads of a 128-row Q^T/K^T
    chunk are packed into the PE array as two K=64 row-tiles (tile_position
    (0,0)/(64,0)) running concurrently.
  - Row sums of exp(scores) come for free from a ones-column appended to V
    (M=65 stationary); normalization uses an fp16 K=1 broadcast matmul +
    DVE fast-reciprocal/multiply, emitted lazily into the next block so the
    in-order PE stream never stalls at block boundaries.
  - Schedule: exp(scores) on ScalarE is the critical engine; the lead
    emits only the 3 Q/K chunks block 0 strictly needs, and all other
    projections/normalization interleave into the attention j-loops at
    one-matmul granularity to keep both PE and ACT dense.
"""

import os
import sys

for _p in ("/opt/trn_rl_repo",):
    if os.path.isdir(_p) and _p not in sys.path:
        sys.path.append(_p)

import ml_dtypes
import numpy as np

import concourse.bass as bass
import concourse.tile as tile
from concourse import bacc, mybir
from concourse.bass_utils import run_bass_kernel_spmd

BF16 = mybir.dt.bfloat16
F16 = mybir.dt.float16
F32 = mybir.dt.float32

B, S, DM = 4, 2048, 512
H, DH = 8, 64
HPC = 4  # heads per core
DQ = HPC * DH  # 256: per-core slice of the inner dim
N_CORES = 8
SCALE = DH**-0.5

AF = mybir.ActivationFunctionType

# exported for test harnesses
LAST_EXEC_TIME_NS = None
LAST_RESULT = None

_CACHED_NC = None


def _kernel_body(tc, xT_d, wq_d, wk_d, wv_d, wo_d, out_d):
    from contextlib import ExitStack

    nc = tc.nc
    with ExitStack() as ctx:
        consts = ctx.enter_context(tc.tile_pool(name="consts", bufs=1))
        ptp = ctx.enter_context(tc.tile_pool(name="pt", bufs=10))
        normp = ctx.enter_context(tc.tile_pool(name="norm", bufs=3))
        foutp = ctx.enter_context(tc.tile_pool(name="fout", bufs=4))
        # PSUM budget (8 banks): "s" 2x[128,1024]=4, "o" 3x[65,512]=3, "x" 1
        ps_s = ctx.enter_context(tc.tile_pool(name="ps_s", bufs=2, space="PSUM"))
        ps_o = ctx.enter_context(tc.tile_pool(name="ps_o", bufs=3, space="PSUM"))
        ps_x = ctx.enter_context(tc.tile_pool(name="ps_x", bufs=1, space="PSUM"))

        sb_xT = consts.tile([128, 4, S], BF16)  # X^T: k-chunk c -> [:, c, :]
        sb_wq = consts.tile([128, 4, DQ], BF16)
        sb_wk = consts.tile([128, 4, DQ], BF16)
        sb_wv = consts.tile([128, 4, DQ], BF16)
        sb_wo = consts.tile([128, 2, DM], BF16)  # d'-chunk p -> [:, p, :]
        sb_qT = consts.tile([128, 2, S], BF16)  # dq-chunk (head pair) p
        sb_kT = consts.tile([128, 2, S], BF16)
        sb_v = consts.tile([128, 16, HPC, 66], BF16)  # V_aug; col 64 = ones
        sb_oT = consts.tile([128, 2, S], BF16)  # normalized O^T
        sb_warm = consts.tile([128, 512], BF16)  # PE warmup fodder
        sb_one = consts.tile([128, 64], F16)  # all-ones (bcast stationary)
        # bottom half of Wo's pair-1 rows re-homed at partitions 0:64 so the
        # last block's head-1 output never needs a cross-partition DMA
        sb_wo2 = consts.tile([64, DM], BF16)
        sb_stage = consts.tile([64, 512], BF16)  # last-block head-1 oT

        # sb_warm via GPSIMD: that queue inits ~1.5us before DVE, so the PE
        # warmup (gated only on this memset) starts correspondingly earlier
        nc.gpsimd.memset(sb_warm[:], 1.0)
        nc.vector.memset(sb_one[:], 1.0)
        nc.vector.memset(sb_v[:, :, :, 64:66], 1.0)
        # Input DMAs: all on ONE queue (splitting across queues just splits
        # the shared ~356GB/s HBM bandwidth and slows the critical prefix).
        # Strict need-order with 512-column X^T slices so the lead matmuls
        # pipeline with the arriving data: Q0c0 is unblocked after ~0.8MB
        # instead of the full 2.8MB.
        xT_r = xT_d.rearrange("(c p) s -> c p s", p=128)
        nc.sync.dma_start(sb_wq[:], wq_d.rearrange("(c p) d -> p c d", p=128))
        for kc in range(4):
            nc.sync.dma_start(sb_xT[:, kc, 0:512], xT_r[kc][:, 0:512])
        nc.sync.dma_start(sb_wk[:], wk_d.rearrange("(c p) d -> p c d", p=128))
        nc.sync.dma_start(sb_wv[:], wv_d.rearrange("(c p) d -> p c d", p=128))
        for kc in range(4):
            nc.sync.dma_start(sb_xT[:, kc, 512:1024], xT_r[kc][:, 512:1024])
        for kc in range(4):
            nc.sync.dma_start(sb_xT[:, kc, 1024:1536], xT_r[kc][:, 1024:1536])
        for kc in range(4):
            nc.sync.dma_start(sb_xT[:, kc, 1536:2048], xT_r[kc][:, 1536:2048])
        nc.sync.dma_start(sb_wo[:], wo_d.rearrange("(c p) d -> p c d", p=128))
        nc.sync.dma_start(sb_wo2[:], wo_d[192:256, :])

        # Warm the PE (HAM un-throttle needs ~3.4us of sustained matmul) and
        # preload the exp table while the first DMAs are in flight; the lead
        # matmuls themselves continue the warmup as data lands.
        pw = ps_x.tile([128, 512], F32, tag="x")
        for r in range(9):
            nc.tensor.matmul(
                pw[:], lhsT=sb_warm[:, 0:128], rhs=sb_warm[:], start=True, stop=True
            )
        warm_act = normp.tile([1, 4], F32, tag="wact")
        nc.scalar.activation(warm_act[:], pw[0:1, 0:4], AF.Exp, scale=-1.0)

        def emit_qk_chunk(w_sb, dst_sb, p, c, pool_tag=("ps_s", "s")):
            """One [128,512] chunk of Q^T or K^T for head-pair p."""
            isl = slice(c * 512, (c + 1) * 512)
            pool = {"ps_s": ps_s, "ps_o": ps_o, "ps_x": ps_x}[pool_tag[0]]
            pq = pool.tile([128, 512], F32, tag=pool_tag[1], name="pqk")
            for kc in range(4):
                nc.tensor.matmul(
                    pq[:],
                    lhsT=w_sb[:, kc, p * 128 : (p + 1) * 128],
                    rhs=sb_xT[:, kc, isl],
                    start=(kc == 0),
                    stop=(kc == 3),
                )
            nc.vector.tensor_copy(dst_sb[:, p, isl], pq[:])

        def emit_qk_chunk_mm(w_sb, p, c, kc, pq):
            nc.tensor.matmul(
                pq[:],
                lhsT=w_sb[:, kc, p * 128 : (p + 1) * 128],
                rhs=sb_xT[:, kc, c * 512 : (c + 1) * 512],
                start=(kc == 0),
                stop=(kc == 3),
            )

        def emit_v_chunk(sc):
            """V natural [s,dv] for s-chunk sc (all 4 heads)."""
            pv = ps_x.tile([128, DQ], F32, tag="x", name="pv")
            for kc in range(4):
                nc.tensor.matmul(
                    pv[:],
                    lhsT=sb_xT[:, kc, sc * 128 : (sc + 1) * 128],
                    rhs=sb_wv[:, kc, :],
                    start=(kc == 0),
                    stop=(kc == 3),
                )
            nc.vector.tensor_copy(
                sb_v[:, sc, :, 0:64], pv.rearrange("p (h d) -> p h d", h=HPC)
            )

        # ---- lead: exactly what block 0 strictly needs, in DMA-arrival
        # order: Q^T/K^T chunk 0 for pair 0, plus V chunks 0-3 (which fill
        # the PE-idle window while the second X^T S-quarter is still
        # landing). Everything else streams into the j-loops. ----
        emit_qk_chunk(sb_wq, sb_qT, 0, 0, ("ps_o", "o"))
        emit_qk_chunk(sb_wk, sb_kT, 0, 0)
        for sc in range(4):
            emit_v_chunk(sc)

        # deferred QK work, flattened to per-MM granularity, in deadline
        # order (Q1c0 is needed by the 2-ahead score stream at iter 62,
        # before K1's chunks 2/3 at iters 70/74)
        pending_qk = [(sb_wq, sb_qT, 0, c) for c in (1, 2, 3)]
        pending_qk += [(sb_wk, sb_kT, 1, 0), (sb_wk, sb_kT, 1, 1)]
        pending_qk.append((sb_wq, sb_qT, 1, 0))
        pending_qk += [(sb_wk, sb_kT, 1, 2), (sb_wk, sb_kT, 1, 3)]
        pending_qk += [(sb_wq, sb_qT, 1, c) for c in (1, 2, 3)]
        qk_state = {"chunk": None, "tile": None, "kc": 0}

        def step_pending_qk():
            stt = qk_state
            if stt["chunk"] is None:
                if not pending_qk:
                    return
                stt["chunk"] = pending_qk.pop(0)
                stt["tile"] = ps_x.tile([128, 512], F32, tag="x", name="pqk1")
                stt["kc"] = 0
            w_sb, dst_sb, p, c = stt["chunk"]
            emit_qk_chunk_mm(w_sb, p, c, stt["kc"], stt["tile"])
            stt["kc"] += 1
            if stt["kc"] == 4:
                nc.vector.tensor_copy(
                    dst_sb[:, p, c * 512 : (c + 1) * 512], stt["tile"][:]
                )
                stt["chunk"] = None

        # ---- attention: pair 0 then pair 1 ----
        # Normalization of block k is emitted lazily, interleaved into the
        # first iterations of block k+1, so the in-order PE stream never
        # stalls long enough for HAM to re-throttle the clock.
        def make_norm_steps(p, ic, po, last=False):
            """Normalization of a finished block, split into 3 steps that the
            next block interleaves into its first iterations (the fp16 K=1
            broadcast matmuls sit behind fresh scores in PE order, so the PE
            never stalls waiting on the DVE sums copies). For the last block,
            head 1's result goes to the partitions-0:63 staging tile (read
            against sb_wo2) instead of the slow cross-partition DMA."""
            isl = slice(ic * 512, (ic + 1) * 512)
            held = {}

            def step_sums():
                for hi in range(2):
                    s = normp.tile([65, 512], F16, tag="sums", name=f"sums{hi}")
                    nc.vector.tensor_copy(s[64:65, :], po[hi][64:65, :])
                    held[hi] = s

            def step_head(hi):
                if last and hi == 1:
                    # tail: the spare "o" slot is free; avoids serializing
                    # behind head 0's pb in the single "x" slot
                    pb = ps_o.tile([64, 512], F32, tag="o", name="pbz")
                else:
                    pb = ps_x.tile([64, 512], F32, tag="x", name=f"pb{hi}")
                nc.tensor.matmul(
                    pb[:],
                    lhsT=sb_one[64:65, :],
                    rhs=held[hi][64:65, :],
                    start=True,
                    stop=True,
                )
                rec = normp.tile([64, 512], F32, tag="rec", name=f"rec{hi}")
                nc.vector.reciprocal_approx_fast(rec[:], pb[:])
                if hi == 0:
                    nc.vector.tensor_mul(
                        sb_oT[0:64, p, isl], po[0][0:64, :], rec[:]
                    )
                elif last:
                    nc.vector.tensor_mul(sb_stage[:], po[1][0:64, :], rec[:])
                else:
                    tmpb = normp.tile([64, 512], BF16, tag="tmpb")
                    nc.vector.tensor_mul(tmpb[:], po[1][0:64, :], rec[:])
                    nc.sync.dma_start(sb_oT[64:128, p, isl], tmpb[:])

            return [step_sums, lambda: step_head(0), lambda: step_head(1)]

        # per-MM-granularity deferred projection chunks (run during p1 blocks)
        pending_proj = []
        proj_state = {"c2": None, "tile": None, "p": 0}

        def step_pending_proj():
            stt = proj_state
            if stt["c2"] is None:
                if not pending_proj:
                    return
                stt["c2"] = pending_proj.pop(0)
                stt["tile"] = ps_x.tile([128, 512], F32, tag="x", name="pf")
                stt["p"] = 0
            c2, p = stt["c2"], stt["p"]
            nc.tensor.matmul(
                stt["tile"][:],
                lhsT=sb_oT[:, p, c2 * 128 : (c2 + 1) * 128],
                rhs=sb_wo[:, p, :],
                start=(p == 0),
                stop=(p == 1),
            )
            stt["p"] += 1
            if stt["p"] == 2:
                fo = foutp.tile([128, 512], F32, tag="fo")
                nc.vector.tensor_copy(fo[:], stt["tile"][:])
                nc.sync.dma_start(out_d[c2 * 128 : (c2 + 1) * 128, :], fo[:])
                stt["c2"] = None

        def step_extras():
            """One deferred matmul: pending QK first, then projections."""
            if pending_qk or qk_state["chunk"] is not None:
                step_pending_qk()
            else:
                step_pending_proj()

        pending_norm = []
        blocks = [(p, ic) for p in range(2) for ic in range(4)]

        def emit_scores(p, ic, j):
            isl = slice(ic * 512, (ic + 1) * 512)
            jsl = slice(j * 128, (j + 1) * 128)
            st = ps_s.tile([128, 1024], F32, tag="s")
            nc.tensor.matmul(
                st[:, 0:512],
                lhsT=sb_kT[0:64, p, jsl],
                rhs=sb_qT[0:64, p, isl],
                start=True,
                stop=True,
            )
            nc.tensor.matmul(
                st[:, 512:1024],
                lhsT=sb_kT[64:128, p, jsl],
                rhs=sb_qT[64:128, p, isl],
                start=True,
                stop=True,
            )
            return st

        def emit_exp(st):
            pt = ptp.tile([128, 1024], BF16, tag="pt")
            nc.scalar.activation(pt[:], st[:], AF.Exp, scale=SCALE)
            return pt

        # Global score/exp stream runs TWO iterations ahead of the AV
        # stream: scores(k+2) issue before iteration k's AVs, so at block
        # boundaries the next block's first exps never queue behind the
        # last AVs (which themselves wait on the final exp of the block).
        # The 2-slot score pool self-paces the stream: scores(k+2) can't
        # run before EXP(k) has drained its slot.
        sched = [(p, ic, j) for (p, ic) in blocks for j in range(16)]
        pts = {}

        def emit_scores_exp(k):
            p, ic, j = sched[k]
            pts[k] = emit_exp(emit_scores(p, ic, j))

        emit_scores_exp(0)
        emit_scores_exp(1)
        po = None
        pending_burst = None

        def make_av(p_, po_):
            def emit_av(hi, jj, ptt):
                nc.tensor.matmul(
                    po_[hi][:],
                    lhsT=sb_v[:, jj, 2 * p_ + hi, 0:65],
                    rhs=ptt[:, hi * 512 : (hi + 1) * 512],
                    start=(jj == 0),
                    stop=(jj == 15),
                    skip_group_check=True,
                )

            return emit_av

        for k, (p, ic, j) in enumerate(sched):
            if j == 0:
                po = [
                    ps_o.tile([65, 512], F32, tag="o", name=f"po{hi}")
                    for hi in range(2)
                ]
                emit_av = make_av(p, po)
                if p == 1 and ic > 0:
                    # previous ic's projection slice; its oT inputs complete
                    # during this block's first two iterations (lazy norm)
                    pending_proj.extend(range(4 * (ic - 1), 4 * ic))
            if k + 2 < len(sched):
                emit_scores_exp(k + 2)
            if pending_burst is not None:
                # previous block's last three AVs + sums, emitted AFTER this
                # iteration's score pair so the next exps never queue behind
                # AVs that themselves wait on the previous block's last exp
                pending_burst()
                pending_burst = None
            if pending_norm:
                if j == 1:
                    pending_norm[0]()  # bcast+recip+mul head 0
                elif j == 2:
                    pending_norm[1]()  # ... head 1
                    pending_norm = []
            # extras: deferred matmuls keep PE fed; x-slot is needed
            # by the norm broadcasts at j=1,2 so extras wait till j>=3
            # (block 1 starts later still: it carries the densest deferred
            # stream and was PE-oversubscribed at j=3,4,5)
            if p == 0 and ic == 0:
                # K chunks 1-3 at 2 matmuls/iter over j=0..5; chunk c's copy
                # lands at j=2c-1, one iter before the 2-ahead score stream
                # for j=4c reads it
                if j <= 5:
                    c, half = 1 + j // 2, j % 2
                    if half == 0:
                        kq_tile = ps_o.tile([128, 512], F32, tag="o", name="pkh")
                    for kc in (0, 1) if half == 0 else (2, 3):
                        emit_qk_chunk_mm(sb_wk, 0, c, kc, kq_tile)
                    if half == 1:
                        nc.vector.tensor_copy(
                            sb_kT[:, 0, c * 512 : (c + 1) * 512], kq_tile[:]
                        )
                if 3 <= j < 15:
                    emit_v_chunk(j + 1)
                if j >= 7:
                    step_extras()  # Q0c1 must complete before iter 14
            elif j >= (6 if (p == 0 and ic == 1) else 5):
                step_extras()

            # uniform AV lag (h0 by 1 iter, h1 by 2) keeps scores ahead
            # of the AV stream so ACT never waits at block boundaries
            if j >= 1:
                emit_av(0, j - 1, pts[k - 1])
            if j >= 2:
                emit_av(1, j - 2, pts[k - 2])
            if j == 15:

                def mk_burst(av_, p_, ic_, po_, k_, last_):
                    def burst():
                        nonlocal pending_norm
                        av_(0, 15, pts[k_])
                        av_(1, 14, pts[k_ - 1])
                        av_(1, 15, pts[k_])
                        pending_norm = make_norm_steps(p_, ic_, po_, last=last_)
                        pending_norm[0]()  # sums copies right behind the AVs
                        pending_norm = pending_norm[1:]

                    return burst

                pending_burst = mk_burst(
                    emit_av, p, ic, po, k, k == len(sched) - 1
                )
        pending_burst()  # last block's AV tail + sums

        # ---- tail: last normalize + remaining projection chunks ----
        # head 0 first: its chain is pure DVE and unblocks the oT top half
        pending_norm[0]()
        pending_norm[1]()
        while pending_proj or proj_state["c2"] is not None:
            step_pending_proj()
        # final four chunks, packed two per score-pool tile (those slots are
        # free the moment the last EXP consumed them): the pair-0 partials
        # depend only on long-finished oT pair-0 rows, so all four issue
        # immediately and overlap the norm chain above. The pair-1 matmuls
        # are split K=64: the top halves run off head 0's DVE-written oT
        # rows; the bottom halves contract the partitions-0:63 staging tile
        # against sb_wo2, so no cross-partition DMA gates the tail.
        # Groups interleave across tiles (skip_group_check).
        pfz = [ps_s.tile([128, 1024], F32, tag="s", name=f"pfz{n}") for n in range(2)]
        slots = [(c2, pfz[n // 2][:, (n % 2) * 512 : (n % 2 + 1) * 512])
                 for n, c2 in enumerate(range(12, 16))]
        for c2, pf in slots:
            nc.tensor.matmul(
                pf,
                lhsT=sb_oT[:, 0, c2 * 128 : (c2 + 1) * 128],
                rhs=sb_wo[:, 0, :],
                start=True,
                stop=False,
                skip_group_check=True,
            )
        for c2, pf in slots:
            nc.tensor.matmul(
                pf,
                lhsT=sb_oT[0:64, 1, c2 * 128 : (c2 + 1) * 128],
                rhs=sb_wo[0:64, 1, :],
                start=False,
                stop=False,
                skip_group_check=True,
            )
        for c2, pf in slots:
            nc.tensor.matmul(
                pf,
                lhsT=sb_stage[:, (c2 - 12) * 128 : (c2 - 11) * 128],
                rhs=sb_wo2[:],
                start=False,
                stop=True,
                skip_group_check=True,
            )
        for c2, pf in slots:
            fo = foutp.tile([128, 512], F32, tag="fo")
            nc.vector.tensor_copy(fo[:], pf)
            nc.sync.dma_start(out_d[c2 * 128 : (c2 + 1) * 128, :], fo[:])


def _build():
    nc = bacc.Bacc("TRN2", target_bir_lowering=False, debug=False, num_devices=N_CORES)
    xT = nc.dram_tensor("xT", [DM, S], BF16, kind="ExternalInput")
    wq = nc.dram_tensor("wq", [DM, DQ], BF16, kind="ExternalInput")
    wk = nc.dram_tensor("wk", [DM, DQ], BF16, kind="ExternalInput")
    wv = nc.dram_tensor("wv", [DM, DQ], BF16, kind="ExternalInput")
    wo = nc.dram_tensor("wo", [DQ, DM], BF16, kind="ExternalInput")
    out = nc.dram_tensor("out", [S, DM], F32, kind="ExternalOutput")
    with tile.TileContext(nc) as tc:
        _kernel_body(tc, xT.ap(), wq.ap(), wk.ap(), wv.ap(), wo.ap(), out.ap())
    nc.compile()
    return nc


def get_nc():
    global _CACHED_NC
    if _CACHED_NC is None:
        _CACHED_NC = _build()
    return _CACHED_NC


def _in_maps(hidden_states, Wq, Wk, Wv, Wo):
    bf = ml_dtypes.bfloat16
    maps = []
    for c in range(N_CORES):
        b, g = c // 2, c % 2
        cols = slice(g * DQ, (g + 1) * DQ)
        maps.append(
            {
                "xT": np.ascontiguousarray(hidden_states[b].T).astype(bf),
                "wq": np.ascontiguousarray(Wq[:, cols]).astype(bf),
                "wk": np.ascontiguousarray(Wk[:, cols]).astype(bf),
                "wv": np.ascontiguousarray(Wv[:, cols]).astype(bf),
                "wo": np.ascontiguousarray(Wo[cols, :]).astype(bf),
            }
        )
    return maps


def _ensure_profile_support():
    """Best-effort: register the axon NTFF profiling hook + defang the
    bucket upload (zero-egress container). Without this, trace=True dies
    on a missing ``antenv.axon_hooks`` module in this image."""
    import types

    try:
        import antenv

        if "antenv.axon_hooks" not in sys.modules:
            mod = types.ModuleType("antenv.axon_hooks")
            _h = {"hook": None}
            mod.set_axon_ntff_profile_hook = lambda h: _h.__setitem__("hook", h)
            mod.get_axon_ntff_profile_hook = lambda: _h["hook"]
            sys.modules["antenv.axon_hooks"] = mod
            antenv.axon_hooks = mod
        import antenv.axon_hooks as ah

        if ah.get_axon_ntff_profile_hook() is None:
            if "/root/.axon_site" not in sys.path:
                sys.path.append("/root/.axon_site")
            from trn_agent_boot.trn_boot import _ntff_profile_via_ctypes

            hook = _ntff_profile_via_ctypes("/opt/axon/libaxon_pjrt.so")
            if hook is not None:
                ah.set_axon_ntff_profile_hook(hook)
    except Exception:
        pass
    try:
        import concourse.bass_utils as bu

        bu.upload_artifacts = lambda tmpdir: tmpdir
    except Exception:
        pass


def kernel(hidden_states, Wq, Wk, Wv, Wo):
    global LAST_EXEC_TIME_NS, LAST_RESULT
    hidden_states = np.asarray(hidden_states, dtype=np.float32)
    Wq, Wk, Wv, Wo = (np.asarray(w, dtype=np.float32) for w in (Wq, Wk, Wv, Wo))

    trace = bool(os.environ.get("BASS_TRACE"))
    if trace:
        _ensure_profile_support()
    nc = get_nc()
    maps = _in_maps(hidden_states, Wq, Wk, Wv, Wo)
    res = run_bass_kernel_spmd(
        nc,
        maps,
        core_ids=list(range(N_CORES)),
        trace=trace,
        tmpdir=os.environ.get("BASS_TRACE_DIR") or None,
    )
    LAST_RESULT = res
    LAST_EXEC_TIME_NS = res.exec_time_ns

    out = np.empty((B, S, DM), dtype=np.float32)
    for b in range(B):
        out[b] = res.results[2 * b]["out"] + res.results[2 * b + 1]["out"]
    return out


if __name__ == "__main__":
    rng = np.random.default_rng(0)
    hs = rng.standard_normal((B, S, DM), dtype=np.float32)
    ws = [
        (rng.standard_normal((DM, DM), dtype=np.float32) / np.sqrt(DM))
        for _ in range(4)
    ]
    o = kernel(hs, *ws)
    print("out", o.shape, o.dtype, float(np.abs(o).mean()))
    print("exec_time_ns", LAST_EXEC_TIME_NS)


# revision 27
# speedup vs baseline: 1.1715x; 1.0008x over previous
"""Multi-head attention (B=4, S=2048, H=8, Dh=64, Dm=512) on 8 TRN2 NeuronCores.

Sharding: batch*head parallel. Core c owns batch b = c//2 and head group
g = c%2 (4 heads each). Each core computes QKV projection for its head
group, transposed-scores flash-style attention (no max subtraction --
scores ~ N(0,1) after 1/sqrt(Dh) scaling, exp is safe in fp32/bf16), and
its partial output projection against its 256 rows of Wo. The host sums
the two partial projections per batch.

Device-side layout notes:
  - X^T (bf16) is prepared on host so every matmul contracts over the
    partition dim directly.
  - Scores are computed transposed (S^T[j,i] = K Q^T) so the attention*V
    matmul needs no transposition; the two heads of a 128-row Q^T/K^T
    chunk are packed into the PE array as two K=64 row-tiles (tile_position
    (0,0)/(64,0)) running concurrently.
  - Row sums of exp(scores) come for free from a ones-column appended to V
    (M=65 stationary); normalization uses an fp16 K=1 broadcast matmul +
    DVE fast-reciprocal/multiply, emitted lazily into the next block so the
    in-order PE stream never stalls at block boundaries.
  - Schedule: exp(scores) on ScalarE is the critical engine; the lead
    emits only the 3 Q/K chunks block 0 strictly needs, and all other
    projections/normalization interleave into the attention j-loops at
    one-matmul granularity to keep both PE and ACT dense.
"""

import os
import sys

for _p in ("/opt/trn_rl_repo",):
    if os.path.isdir(_p) and _p not in sys.path:
        sys.path.append(_p)

import ml_dtypes
import numpy as np

import concourse.bass as bass
import concourse.tile as tile
from concourse import bacc, mybir
from concourse.bass_utils import run_bass_kernel_spmd

BF16 = mybir.dt.bfloat16
F16 = mybir.dt.float16
F32 = mybir.dt.float32

B, S, DM = 4, 2048, 512
H, DH = 8, 64
HPC = 4  # heads per core
DQ = HPC * DH  # 256: per-core slice of the inner dim
N_CORES = 8
SCALE = DH**-0.5

AF = mybir.ActivationFunctionType

# exported for test harnesses
LAST_EXEC_TIME_NS = None
LAST_RESULT = None

_CACHED_NC = None


def _kernel_body(tc, xT_d, wq_d, wk_d, wv_d, wo_d, out_d):
    from contextlib import ExitStack

    nc = tc.nc
    with ExitStack() as ctx:
        consts = ctx.enter_context(tc.tile_pool(name="consts", bufs=1))
        ptp = ctx.enter_context(tc.tile_pool(name="pt", bufs=10))
        normp = ctx.enter_context(tc.tile_pool(name="norm", bufs=3))
        foutp = ctx.enter_context(tc.tile_pool(name="fout", bufs=4))
        # PSUM budget (8 banks): "s" 2x[128,1024]=4, "o" 3x[65,512]=3, "x" 1
        ps_s = ctx.enter_context(tc.tile_pool(name="ps_s", bufs=2, space="PSUM"))
        ps_o = ctx.enter_context(tc.tile_pool(name="ps_o", bufs=3, space="PSUM"))
        ps_x = ctx.enter_context(tc.tile_pool(name="ps_x", bufs=1, space="PSUM"))

        sb_xT = consts.tile([128, 4, S], BF16)  # X^T: k-chunk c -> [:, c, :]
        sb_wq = consts.tile([128, 4, DQ], BF16)
        sb_wk = consts.tile([128, 4, DQ], BF16)
        sb_wv = consts.tile([128, 4, DQ], BF16)
        sb_wo = consts.tile([128, 2, DM], BF16)  # d'-chunk p -> [:, p, :]
        sb_qT = consts.tile([128, 2, S], BF16)  # dq-chunk (head pair) p
        sb_kT = consts.tile([128, 2, S], BF16)
        sb_v = consts.tile([128, 16, HPC, 66], BF16)  # V_aug; col 64 = ones
        sb_oT = consts.tile([128, 2, S], BF16)  # normalized O^T
        sb_warm = consts.tile([128, 512], BF16)  # PE warmup fodder
        sb_one = consts.tile([128, 64], F16)  # all-ones (bcast stationary)
        # bottom half of Wo's pair-1 rows re-homed at partitions 0:64 so the
        # last block's head-1 output never needs a cross-partition DMA
        sb_wo2 = consts.tile([64, DM], BF16)
        sb_stage = consts.tile([64, 512], BF16)  # last-block head-1 oT

        # sb_warm via GPSIMD: that queue inits ~1.5us before DVE, so the PE
        # warmup (gated only on this memset) starts correspondingly earlier
        nc.gpsimd.memset(sb_warm[:], 1.0)
        nc.vector.memset(sb_one[:], 1.0)
        nc.vector.memset(sb_v[:, :, :, 64:66], 1.0)
        # Input DMAs: all on ONE queue (splitting across queues just splits
        # the shared ~356GB/s HBM bandwidth and slows the critical prefix).
        # Strict need-order with 512-column X^T slices so the lead matmuls
        # pipeline with the arriving data: Q0c0 is unblocked after ~0.8MB
        # instead of the full 2.8MB.
        xT_r = xT_d.rearrange("(c p) s -> c p s", p=128)
        nc.sync.dma_start(sb_wq[:], wq_d.rearrange("(c p) d -> p c d", p=128))
        for kc in range(4):
            nc.sync.dma_start(sb_xT[:, kc, 0:512], xT_r[kc][:, 0:512])
        nc.sync.dma_start(sb_wk[:], wk_d.rearrange("(c p) d -> p c d", p=128))
        nc.sync.dma_start(sb_wv[:], wv_d.rearrange("(c p) d -> p c d", p=128))
        for kc in range(4):
            nc.sync.dma_start(sb_xT[:, kc, 512:1024], xT_r[kc][:, 512:1024])
        for kc in range(4):
            nc.sync.dma_start(sb_xT[:, kc, 1024:1536], xT_r[kc][:, 1024:1536])
        for kc in range(4):
            nc.sync.dma_start(sb_xT[:, kc, 1536:2048], xT_r[kc][:, 1536:2048])
        nc.sync.dma_start(sb_wo[:], wo_d.rearrange("(c p) d -> p c d", p=128))
        nc.sync.dma_start(sb_wo2[:], wo_d[192:256, :])

        # Warm the PE (HAM un-throttle needs ~3.4us of sustained matmul) and
        # preload the exp table while the first DMAs are in flight; the lead
        # matmuls themselves continue the warmup as data lands.
        pw = ps_x.tile([128, 512], F32, tag="x")
        for r in range(9):
            nc.tensor.matmul(
                pw[:], lhsT=sb_warm[:, 0:128], rhs=sb_warm[:], start=True, stop=True
            )
        warm_act = normp.tile([1, 4], F32, tag="wact")
        nc.scalar.activation(warm_act[:], pw[0:1, 0:4], AF.Exp, scale=-1.0)

        def emit_qk_chunk(w_sb, dst_sb, p, c, pool_tag=("ps_s", "s")):
            """One [128,512] chunk of Q^T or K^T for head-pair p."""
            isl = slice(c * 512, (c + 1) * 512)
            pool = {"ps_s": ps_s, "ps_o": ps_o, "ps_x": ps_x}[pool_tag[0]]
            pq = pool.tile([128, 512], F32, tag=pool_tag[1], name="pqk")
            for kc in range(4):
                nc.tensor.matmul(
                    pq[:],
                    lhsT=w_sb[:, kc, p * 128 : (p + 1) * 128],
                    rhs=sb_xT[:, kc, isl],
                    start=(kc == 0),
                    stop=(kc == 3),
                )
            nc.vector.tensor_copy(dst_sb[:, p, isl], pq[:])

        def emit_qk_chunk_mm(w_sb, p, c, kc, pq):
            nc.tensor.matmul(
                pq[:],
                lhsT=w_sb[:, kc, p * 128 : (p + 1) * 128],
                rhs=sb_xT[:, kc, c * 512 : (c + 1) * 512],
                start=(kc == 0),
                stop=(kc == 3),
            )

        def emit_v_chunk(sc):
            """V natural [s,dv] for s-chunk sc (all 4 heads)."""
            pv = ps_x.tile([128, DQ], F32, tag="x", name="pv")
            for kc in range(4):
                nc.tensor.matmul(
                    pv[:],
                    lhsT=sb_xT[:, kc, sc * 128 : (sc + 1) * 128],
                    rhs=sb_wv[:, kc, :],
                    start=(kc == 0),
                    stop=(kc == 3),
                )
            nc.vector.tensor_copy(
                sb_v[:, sc, :, 0:64], pv.rearrange("p (h d) -> p h d", h=HPC)
            )

        # ---- lead: exactly what block 0 strictly needs, in DMA-arrival
        # order: Q^T/K^T chunk 0 for pair 0, plus V chunks 0-3 (which fill
        # the PE-idle window while the second X^T S-quarter is still
        # landing). Everything else streams into the j-loops. ----
        emit_qk_chunk(sb_wq, sb_qT, 0, 0, ("ps_o", "o"))
        emit_qk_chunk(sb_wk, sb_kT, 0, 0)
        for sc in range(4):
            emit_v_chunk(sc)
        emit_qk_chunk(sb_wq, sb_qT, 0, 1, ("ps_o", "o"))

        # deferred QK work, flattened to per-MM granularity, in deadline
        # order (Q1c0 is needed by the 2-ahead score stream at iter 62,
        # before K1's chunks 2/3 at iters 70/74)
        pending_qk = [(sb_wq, sb_qT, 0, c) for c in (2, 3)]
        pending_qk += [(sb_wk, sb_kT, 1, 0), (sb_wk, sb_kT, 1, 1)]
        pending_qk.append((sb_wq, sb_qT, 1, 0))
        pending_qk += [(sb_wk, sb_kT, 1, 2), (sb_wk, sb_kT, 1, 3)]
        pending_qk += [(sb_wq, sb_qT, 1, c) for c in (1, 2, 3)]
        qk_state = {"chunk": None, "tile": None, "kc": 0}

        def step_pending_qk():
            stt = qk_state
            if stt["chunk"] is None:
                if not pending_qk:
                    return
                stt["chunk"] = pending_qk.pop(0)
                stt["tile"] = ps_x.tile([128, 512], F32, tag="x", name="pqk1")
                stt["kc"] = 0
            w_sb, dst_sb, p, c = stt["chunk"]
            emit_qk_chunk_mm(w_sb, p, c, stt["kc"], stt["tile"])
            stt["kc"] += 1
            if stt["kc"] == 4:
                nc.vector.tensor_copy(
                    dst_sb[:, p, c * 512 : (c + 1) * 512], stt["tile"][:]
                )
                stt["chunk"] = None

        # ---- attention: pair 0 then pair 1 ----
        # Normalization of block k is emitted lazily, interleaved into the
        # first iterations of block k+1, so the in-order PE stream never
        # stalls long enough for HAM to re-throttle the clock.
        def make_norm_steps(p, ic, po, last=False):
            """Normalization of a finished block, split into 3 steps that the
            next block interleaves into its first iterations (the fp16 K=1
            broadcast matmuls sit behind fresh scores in PE order, so the PE
            never stalls waiting on the DVE sums copies). For the last block,
            head 1's result goes to the partitions-0:63 staging tile (read
            against sb_wo2) instead of the slow cross-partition DMA."""
            isl = slice(ic * 512, (ic + 1) * 512)
            held = {}

            def step_sums():
                for hi in range(2):
                    s = normp.tile([65, 512], F16, tag="sums", name=f"sums{hi}")
                    nc.vector.tensor_copy(s[64:65, :], po[hi][64:65, :])
                    held[hi] = s

            def step_head(hi):
                if last and hi == 1:
                    # tail: the spare "o" slot is free; avoids serializing
                    # behind head 0's pb in the single "x" slot
                    pb = ps_o.tile([64, 512], F32, tag="o", name="pbz")
                else:
                    pb = ps_x.tile([64, 512], F32, tag="x", name=f"pb{hi}")
                nc.tensor.matmul(
                    pb[:],
                    lhsT=sb_one[64:65, :],
                    rhs=held[hi][64:65, :],
                    start=True,
                    stop=True,
                )
                rec = normp.tile([64, 512], F32, tag="rec", name=f"rec{hi}")
                nc.vector.reciprocal_approx_fast(rec[:], pb[:])
                if hi == 0:
                    nc.vector.tensor_mul(
                        sb_oT[0:64, p, isl], po[0][0:64, :], rec[:]
                    )
                elif last:
                    nc.vector.tensor_mul(sb_stage[:], po[1][0:64, :], rec[:])
                else:
                    tmpb = normp.tile([64, 512], BF16, tag="tmpb")
                    nc.vector.tensor_mul(tmpb[:], po[1][0:64, :], rec[:])
                    nc.sync.dma_start(sb_oT[64:128, p, isl], tmpb[:])

            return [step_sums, lambda: step_head(0), lambda: step_head(1)]

        # per-MM-granularity deferred projection chunks (run during p1 blocks)
        pending_proj = []
        proj_state = {"c2": None, "tile": None, "p": 0}

        def step_pending_proj():
            stt = proj_state
            if stt["c2"] is None:
                if not pending_proj:
                    return
                stt["c2"] = pending_proj.pop(0)
                stt["tile"] = ps_x.tile([128, 512], F32, tag="x", name="pf")
                stt["p"] = 0
            c2, p = stt["c2"], stt["p"]
            nc.tensor.matmul(
                stt["tile"][:],
                lhsT=sb_oT[:, p, c2 * 128 : (c2 + 1) * 128],
                rhs=sb_wo[:, p, :],
                start=(p == 0),
                stop=(p == 1),
            )
            stt["p"] += 1
            if stt["p"] == 2:
                fo = foutp.tile([128, 512], F32, tag="fo")
                nc.vector.tensor_copy(fo[:], stt["tile"][:])
                nc.sync.dma_start(out_d[c2 * 128 : (c2 + 1) * 128, :], fo[:])
                stt["c2"] = None

        def step_extras():
            """One deferred matmul: pending QK first, then projections."""
            if pending_qk or qk_state["chunk"] is not None:
                step_pending_qk()
            else:
                step_pending_proj()

        pending_norm = []
        blocks = [(p, ic) for p in range(2) for ic in range(4)]

        def emit_scores(p, ic, j):
            isl = slice(ic * 512, (ic + 1) * 512)
            jsl = slice(j * 128, (j + 1) * 128)
            st = ps_s.tile([128, 1024], F32, tag="s")
            nc.tensor.matmul(
                st[:, 0:512],
                lhsT=sb_kT[0:64, p, jsl],
                rhs=sb_qT[0:64, p, isl],
                start=True,
                stop=True,
            )
            nc.tensor.matmul(
                st[:, 512:1024],
                lhsT=sb_kT[64:128, p, jsl],
                rhs=sb_qT[64:128, p, isl],
                start=True,
                stop=True,
            )
            return st

        def emit_exp(st):
            pt = ptp.tile([128, 1024], BF16, tag="pt")
            nc.scalar.activation(pt[:], st[:], AF.Exp, scale=SCALE)
            return pt

        # Global score/exp stream runs TWO iterations ahead of the AV
        # stream: scores(k+2) issue before iteration k's AVs, so at block
        # boundaries the next block's first exps never queue behind the
        # last AVs (which themselves wait on the final exp of the block).
        # The 2-slot score pool self-paces the stream: scores(k+2) can't
        # run before EXP(k) has drained its slot.
        sched = [(p, ic, j) for (p, ic) in blocks for j in range(16)]
        pts = {}

        def emit_scores_exp(k):
            p, ic, j = sched[k]
            pts[k] = emit_exp(emit_scores(p, ic, j))

        emit_scores_exp(0)
        emit_scores_exp(1)
        po = None
        pending_burst = None

        def make_av(p_, po_):
            def emit_av(hi, jj, ptt):
                nc.tensor.matmul(
                    po_[hi][:],
                    lhsT=sb_v[:, jj, 2 * p_ + hi, 0:65],
                    rhs=ptt[:, hi * 512 : (hi + 1) * 512],
                    start=(jj == 0),
                    stop=(jj == 15),
                    skip_group_check=True,
                )

            return emit_av

        for k, (p, ic, j) in enumerate(sched):
            if j == 0:
                po = [
                    ps_o.tile([65, 512], F32, tag="o", name=f"po{hi}")
                    for hi in range(2)
                ]
                emit_av = make_av(p, po)
                if p == 1 and ic > 0:
                    # previous ic's projection slice; its oT inputs complete
                    # during this block's first two iterations (lazy norm)
                    pending_proj.extend(range(4 * (ic - 1), 4 * ic))
            if k + 2 < len(sched):
                emit_scores_exp(k + 2)
            if pending_burst is not None:
                # previous block's last three AVs + sums, emitted AFTER this
                # iteration's score pair so the next exps never queue behind
                # AVs that themselves wait on the previous block's last exp
                pending_burst()
                pending_burst = None
            if pending_norm:
                if j == 1:
                    pending_norm[0]()  # bcast+recip+mul head 0
                elif j == 2:
                    pending_norm[1]()  # ... head 1
                    pending_norm = []
            # extras: deferred matmuls keep PE fed; x-slot is needed
            # by the norm broadcasts at j=1,2 so extras wait till j>=3
            # (block 1 starts later still: it carries the densest deferred
            # stream and was PE-oversubscribed at j=3,4,5)
            if p == 0 and ic == 0:
                # K chunks 1-3 at 2 matmuls/iter over j=0..5; chunk c's copy
                # lands at j=2c-1, one iter before the 2-ahead score stream
                # for j=4c reads it
                if j <= 5:
                    c, half = 1 + j // 2, j % 2
                    if half == 0:
                        kq_tile = ps_o.tile([128, 512], F32, tag="o", name="pkh")
                    for kc in (0, 1) if half == 0 else (2, 3):
                        emit_qk_chunk_mm(sb_wk, 0, c, kc, kq_tile)
                    if half == 1:
                        nc.vector.tensor_copy(
                            sb_kT[:, 0, c * 512 : (c + 1) * 512], kq_tile[:]
                        )
                if 3 <= j < 15:
                    emit_v_chunk(j + 1)
            elif j >= (6 if (p == 0 and ic == 1) else 5):
                step_extras()

            # uniform AV lag (h0 by 1 iter, h1 by 2) keeps scores ahead
            # of the AV stream so ACT never waits at block boundaries
            if j >= 1:
                emit_av(0, j - 1, pts[k - 1])
            if j >= 2:
                emit_av(1, j - 2, pts[k - 2])
            if j == 15:

                def mk_burst(av_, p_, ic_, po_, k_, last_):
                    def burst():
                        nonlocal pending_norm
                        av_(0, 15, pts[k_])
                        av_(1, 14, pts[k_ - 1])
                        av_(1, 15, pts[k_])
                        pending_norm = make_norm_steps(p_, ic_, po_, last=last_)
                        pending_norm[0]()  # sums copies right behind the AVs
                        pending_norm = pending_norm[1:]

                    return burst

                pending_burst = mk_burst(
                    emit_av, p, ic, po, k, k == len(sched) - 1
                )
        pending_burst()  # last block's AV tail + sums

        # ---- tail: last normalize + remaining projection chunks ----
        # head 0 first: its chain is pure DVE and unblocks the oT top half
        pending_norm[0]()
        pending_norm[1]()
        while pending_proj or proj_state["c2"] is not None:
            step_pending_proj()
        # final four chunks, packed two per score-pool tile (those slots are
        # free the moment the last EXP consumed them): the pair-0 partials
        # depend only on long-finished oT pair-0 rows, so all four issue
        # immediately and overlap the norm chain above. The pair-1 matmuls
        # are split K=64: the top halves run off head 0's DVE-written oT
        # rows; the bottom halves contract the partitions-0:63 staging tile
        # against sb_wo2, so no cross-partition DMA gates the tail.
        # Groups interleave across tiles (skip_group_check).
        pfz = [ps_s.tile([128, 1024], F32, tag="s", name=f"pfz{n}") for n in range(2)]
        slots = [(c2, pfz[n // 2][:, (n % 2) * 512 : (n % 2 + 1) * 512])
                 for n, c2 in enumerate(range(12, 16))]
        for c2, pf in slots:
            nc.tensor.matmul(
                pf,
                lhsT=sb_oT[:, 0, c2 * 128 : (c2 + 1) * 128],
                rhs=sb_wo[:, 0, :],
                start=True,
                stop=False,
                skip_group_check=True,
            )
        for c2, pf in slots:
            nc.tensor.matmul(
                pf,
                lhsT=sb_oT[0:64, 1, c2 * 128 : (c2 + 1) * 128],
                rhs=sb_wo[0:64, 1, :],
                start=False,
                stop=False,
                skip_group_check=True,
            )
        for c2, pf in slots:
            nc.tensor.matmul(
                pf,
                lhsT=sb_stage[:, (c2 - 12) * 128 : (c2 - 11) * 128],
                rhs=sb_wo2[:],
                start=False,
                stop=True,
                skip_group_check=True,
            )
        for c2, pf in slots:
            fo = foutp.tile([128, 512], F32, tag="fo")
            nc.vector.tensor_copy(fo[:], pf)
            nc.sync.dma_start(out_d[c2 * 128 : (c2 + 1) * 128, :], fo[:])


def _build():
    nc = bacc.Bacc("TRN2", target_bir_lowering=False, debug=False, num_devices=N_CORES)
    xT = nc.dram_tensor("xT", [DM, S], BF16, kind="ExternalInput")
    wq = nc.dram_tensor("wq", [DM, DQ], BF16, kind="ExternalInput")
    wk = nc.dram_tensor("wk", [DM, DQ], BF16, kind="ExternalInput")
    wv = nc.dram_tensor("wv", [DM, DQ], BF16, kind="ExternalInput")
    wo = nc.dram_tensor("wo", [DQ, DM], BF16, kind="ExternalInput")
    out = nc.dram_tensor("out", [S, DM], F32, kind="ExternalOutput")
    with tile.TileContext(nc) as tc:
        _kernel_body(tc, xT.ap(), wq.ap(), wk.ap(), wv.ap(), wo.ap(), out.ap())
    nc.compile()
    return nc


def get_nc():
    global _CACHED_NC
    if _CACHED_NC is None:
        _CACHED_NC = _build()
    return _CACHED_NC


def _in_maps(hidden_states, Wq, Wk, Wv, Wo):
    bf = ml_dtypes.bfloat16
    maps = []
    for c in range(N_CORES):
        b, g = c // 2, c % 2
        cols = slice(g * DQ, (g + 1) * DQ)
        maps.append(
            {
                "xT": np.ascontiguousarray(hidden_states[b].T).astype(bf),
                "wq": np.ascontiguousarray(Wq[:, cols]).astype(bf),
                "wk": np.ascontiguousarray(Wk[:, cols]).astype(bf),
                "wv": np.ascontiguousarray(Wv[:, cols]).astype(bf),
                "wo": np.ascontiguousarray(Wo[cols, :]).astype(bf),
            }
        )
    return maps


def _ensure_profile_support():
    """Best-effort: register the axon NTFF profiling hook + defang the
    bucket upload (zero-egress container). Without this, trace=True dies
    on a missing ``antenv.axon_hooks`` module in this image."""
    import types

    try:
        import antenv

        if "antenv.axon_hooks" not in sys.modules:
            mod = types.ModuleType("antenv.axon_hooks")
            _h = {"hook": None}
            mod.set_axon_ntff_profile_hook = lambda h: _h.__setitem__("hook", h)
            mod.get_axon_ntff_profile_hook = lambda: _h["hook"]
            sys.modules["antenv.axon_hooks"] = mod
            antenv.axon_hooks = mod
        import antenv.axon_hooks as ah

        if ah.get_axon_ntff_profile_hook() is None:
            if "/root/.axon_site" not in sys.path:
                sys.path.append("/root/.axon_site")
            from trn_agent_boot.trn_boot import _ntff_profile_via_ctypes

            hook = _ntff_profile_via_ctypes("/opt/axon/libaxon_pjrt.so")
            if hook is not None:
                ah.set_axon_ntff_profile_hook(hook)
    except Exception:
        pass
    try:
        import concourse.bass_utils as bu

        bu.upload_artifacts = lambda tmpdir: tmpdir
    except Exception:
        pass


def kernel(hidden_states, Wq, Wk, Wv, Wo):
    global LAST_EXEC_TIME_NS, LAST_RESULT
    hidden_states = np.asarray(hidden_states, dtype=np.float32)
    Wq, Wk, Wv, Wo = (np.asarray(w, dtype=np.float32) for w in (Wq, Wk, Wv, Wo))

    trace = bool(os.environ.get("BASS_TRACE"))
    if trace:
        _ensure_profile_support()
    nc = get_nc()
    maps = _in_maps(hidden_states, Wq, Wk, Wv, Wo)
    res = run_bass_kernel_spmd(
        nc,
        maps,
        core_ids=list(range(N_CORES)),
        trace=trace,
        tmpdir=os.environ.get("BASS_TRACE_DIR") or None,
    )
    LAST_RESULT = res
    LAST_EXEC_TIME_NS = res.exec_time_ns

    out = np.empty((B, S, DM), dtype=np.float32)
    for b in range(B):
        out[b] = res.results[2 * b]["out"] + res.results[2 * b + 1]["out"]
    return out


if __name__ == "__main__":
    rng = np.random.default_rng(0)
    hs = rng.standard_normal((B, S, DM), dtype=np.float32)
    ws = [
        (rng.standard_normal((DM, DM), dtype=np.float32) / np.sqrt(DM))
        for _ in range(4)
    ]
    o = kernel(hs, *ws)
    print("out", o.shape, o.dtype, float(np.abs(o).mean()))
    print("exec_time_ns", LAST_EXEC_TIME_NS)


# revision 29
# speedup vs baseline: 1.1776x; 1.0052x over previous
"""Multi-head attention (B=4, S=2048, H=8, Dh=64, Dm=512) on 8 TRN2 NeuronCores.

Sharding: batch*head parallel. Core c owns batch b = c//2 and head group
g = c%2 (4 heads each). Each core computes QKV projection for its head
group, transposed-scores flash-style attention (no max subtraction --
scores ~ N(0,1) after 1/sqrt(Dh) scaling, exp is safe in fp32/bf16), and
its partial output projection against its 256 rows of Wo. The host sums
the two partial projections per batch.

Device-side layout notes:
  - X^T (bf16) is prepared on host so every matmul contracts over the
    partition dim directly.
  - Scores are computed transposed (S^T[j,i] = K Q^T) so the attention*V
    matmul needs no transposition; the two heads of a 128-row Q^T/K^T
    chunk are packed into the PE array as two K=64 row-tiles (tile_position
    (0,0)/(64,0)) running concurrently.
  - Row sums of exp(scores) come for free from a ones-column appended to V
    (M=65 stationary); normalization uses an fp16 K=1 broadcast matmul +
    DVE fast-reciprocal/multiply, emitted lazily into the next block so the
    in-order PE stream never stalls at block boundaries.
  - Schedule: exp(scores) on ScalarE is the critical engine; the lead
    emits only the 3 Q/K chunks block 0 strictly needs, and all other
    projections/normalization interleave into the attention j-loops at
    one-matmul granularity to keep both PE and ACT dense.
"""

import os
import sys

for _p in ("/opt/trn_rl_repo",):
    if os.path.isdir(_p) and _p not in sys.path:
        sys.path.append(_p)

import ml_dtypes
import numpy as np

import concourse.bass as bass
import concourse.tile as tile
from concourse import bacc, mybir
from concourse.bass_utils import run_bass_kernel_spmd

BF16 = mybir.dt.bfloat16
F16 = mybir.dt.float16
F32 = mybir.dt.float32

B, S, DM = 4, 2048, 512
H, DH = 8, 64
HPC = 4  # heads per core
DQ = HPC * DH  # 256: per-core slice of the inner dim
N_CORES = 8
SCALE = DH**-0.5

AF = mybir.ActivationFunctionType

# exported for test harnesses
LAST_EXEC_TIME_NS = None
LAST_RESULT = None

_CACHED_NC = None


def _kernel_body(tc, xT_d, wq_d, wk_d, wv_d, wo_d, out_d):
    from contextlib import ExitStack

    nc = tc.nc
    with ExitStack() as ctx:
        consts = ctx.enter_context(tc.tile_pool(name="consts", bufs=1))
        ptp = ctx.enter_context(tc.tile_pool(name="pt", bufs=10))
        normp = ctx.enter_context(tc.tile_pool(name="norm", bufs=3))
        foutp = ctx.enter_context(tc.tile_pool(name="fout", bufs=4))
        # PSUM budget (8 banks): "s" 2x[128,1024]=4, "o" 3x[65,512]=3, "x" 1
        ps_s = ctx.enter_context(tc.tile_pool(name="ps_s", bufs=2, space="PSUM"))
        ps_o = ctx.enter_context(tc.tile_pool(name="ps_o", bufs=3, space="PSUM"))
        ps_x = ctx.enter_context(tc.tile_pool(name="ps_x", bufs=1, space="PSUM"))

        sb_xT = consts.tile([128, 4, S], BF16)  # X^T: k-chunk c -> [:, c, :]
        sb_wq = consts.tile([128, 4, DQ], BF16)
        sb_wk = consts.tile([128, 4, DQ], BF16)
        sb_wv = consts.tile([128, 4, DQ], BF16)
        sb_wo = consts.tile([128, 2, DM], BF16)  # d'-chunk p -> [:, p, :]
        sb_qT = consts.tile([128, 2, S], BF16)  # dq-chunk (head pair) p
        sb_kT = consts.tile([128, 2, S], BF16)
        sb_v = consts.tile([128, 16, HPC, 66], BF16)  # V_aug; col 64 = ones
        sb_oT = consts.tile([128, 2, S], BF16)  # normalized O^T
        sb_warm = consts.tile([128, 512], BF16)  # PE warmup fodder
        sb_one = consts.tile([128, 64], F16)  # all-ones (bcast stationary)
        # bottom half of Wo's pair-1 rows re-homed at partitions 0:64 so the
        # last block's head-1 output never needs a cross-partition DMA
        sb_wo2 = consts.tile([64, DM], BF16)
        sb_stage = consts.tile([64, 512], BF16)  # last-block head-1 oT

        # sb_warm via GPSIMD: that queue inits ~1.5us before DVE, so the PE
        # warmup (gated only on this memset) starts correspondingly earlier
        nc.gpsimd.memset(sb_warm[:], 1.0)
        nc.vector.memset(sb_one[:], 1.0)
        nc.vector.memset(sb_v[:, :, :, 64:66], 1.0)
        # Input DMAs: all on ONE queue (splitting across queues just splits
        # the shared ~356GB/s HBM bandwidth and slows the critical prefix).
        # Strict need-order with 512-column X^T slices so the lead matmuls
        # pipeline with the arriving data: Q0c0 is unblocked after ~0.8MB
        # instead of the full 2.8MB.
        xT_r = xT_d.rearrange("(c p) s -> c p s", p=128)
        nc.sync.dma_start(sb_wq[:], wq_d.rearrange("(c p) d -> p c d", p=128))
        for kc in range(4):
            nc.sync.dma_start(sb_xT[:, kc, 0:512], xT_r[kc][:, 0:512])
        nc.sync.dma_start(sb_wk[:], wk_d.rearrange("(c p) d -> p c d", p=128))
        for kc in range(4):
            nc.sync.dma_start(sb_xT[:, kc, 512:1024], xT_r[kc][:, 512:1024])
        nc.sync.dma_start(sb_wv[:], wv_d.rearrange("(c p) d -> p c d", p=128))
        for kc in range(4):
            nc.sync.dma_start(sb_xT[:, kc, 1024:1536], xT_r[kc][:, 1024:1536])
        for kc in range(4):
            nc.sync.dma_start(sb_xT[:, kc, 1536:2048], xT_r[kc][:, 1536:2048])
        nc.sync.dma_start(sb_wo[:], wo_d.rearrange("(c p) d -> p c d", p=128))
        nc.sync.dma_start(sb_wo2[:], wo_d[192:256, :])

        # Warm the PE (HAM un-throttle needs ~3.4us of sustained matmul) and
        # preload the exp table while the first DMAs are in flight; the lead
        # matmuls themselves continue the warmup as data lands.
        pw = ps_x.tile([128, 512], F32, tag="x")
        for r in range(9):
            nc.tensor.matmul(
                pw[:], lhsT=sb_warm[:, 0:128], rhs=sb_warm[:], start=True, stop=True
            )
        warm_act = normp.tile([1, 4], F32, tag="wact")
        nc.scalar.activation(warm_act[:], pw[0:1, 0:4], AF.Exp, scale=-1.0)

        def emit_qk_chunk(w_sb, dst_sb, p, c, pool_tag=("ps_s", "s")):
            """One [128,512] chunk of Q^T or K^T for head-pair p."""
            isl = slice(c * 512, (c + 1) * 512)
            pool = {"ps_s": ps_s, "ps_o": ps_o, "ps_x": ps_x}[pool_tag[0]]
            pq = pool.tile([128, 512], F32, tag=pool_tag[1], name="pqk")
            for kc in range(4):
                nc.tensor.matmul(
                    pq[:],
                    lhsT=w_sb[:, kc, p * 128 : (p + 1) * 128],
                    rhs=sb_xT[:, kc, isl],
                    start=(kc == 0),
                    stop=(kc == 3),
                )
            nc.vector.tensor_copy(dst_sb[:, p, isl], pq[:])

        def emit_qk_chunk_mm(w_sb, p, c, kc, pq):
            nc.tensor.matmul(
                pq[:],
                lhsT=w_sb[:, kc, p * 128 : (p + 1) * 128],
                rhs=sb_xT[:, kc, c * 512 : (c + 1) * 512],
                start=(kc == 0),
                stop=(kc == 3),
            )

        def emit_v_chunk(sc):
            """V natural [s,dv] for s-chunk sc (all 4 heads)."""
            pv = ps_x.tile([128, DQ], F32, tag="x", name="pv")
            for kc in range(4):
                nc.tensor.matmul(
                    pv[:],
                    lhsT=sb_xT[:, kc, sc * 128 : (sc + 1) * 128],
                    rhs=sb_wv[:, kc, :],
                    start=(kc == 0),
                    stop=(kc == 3),
                )
            nc.vector.tensor_copy(
                sb_v[:, sc, :, 0:64], pv.rearrange("p (h d) -> p h d", h=HPC)
            )

        # ---- lead: exactly what block 0 strictly needs, in DMA-arrival
        # order: Q^T/K^T chunk 0 for pair 0, plus V chunks 0-3 (which fill
        # the PE-idle window while the second X^T S-quarter is still
        # landing). Everything else streams into the j-loops. ----
        emit_qk_chunk(sb_wq, sb_qT, 0, 0, ("ps_o", "o"))
        emit_qk_chunk(sb_wk, sb_kT, 0, 0)
        emit_qk_chunk(sb_wq, sb_qT, 0, 1, ("ps_o", "o"))
        for sc in range(4):
            emit_v_chunk(sc)

        # deferred QK work, flattened to per-MM granularity, in deadline
        # order (Q1c0 is needed by the 2-ahead score stream at iter 62,
        # before K1's chunks 2/3 at iters 70/74)
        pending_qk = [(sb_wq, sb_qT, 0, c) for c in (2, 3)]
        pending_qk += [(sb_wk, sb_kT, 1, 0), (sb_wk, sb_kT, 1, 1)]
        pending_qk.append((sb_wq, sb_qT, 1, 0))
        pending_qk += [(sb_wk, sb_kT, 1, 2), (sb_wk, sb_kT, 1, 3)]
        pending_qk += [(sb_wq, sb_qT, 1, c) for c in (1, 2, 3)]
        qk_state = {"chunk": None, "tile": None, "kc": 0}

        def step_pending_qk():
            stt = qk_state
            if stt["chunk"] is None:
                if not pending_qk:
                    return
                stt["chunk"] = pending_qk.pop(0)
                stt["tile"] = ps_x.tile([128, 512], F32, tag="x", name="pqk1")
                stt["kc"] = 0
            w_sb, dst_sb, p, c = stt["chunk"]
            emit_qk_chunk_mm(w_sb, p, c, stt["kc"], stt["tile"])
            stt["kc"] += 1
            if stt["kc"] == 4:
                nc.vector.tensor_copy(
                    dst_sb[:, p, c * 512 : (c + 1) * 512], stt["tile"][:]
                )
                stt["chunk"] = None

        # ---- attention: pair 0 then pair 1 ----
        # Normalization of block k is emitted lazily, interleaved into the
        # first iterations of block k+1, so the in-order PE stream never
        # stalls long enough for HAM to re-throttle the clock.
        def make_norm_steps(p, ic, po, last=False):
            """Normalization of a finished block, split into 3 steps that the
            next block interleaves into its first iterations (the fp16 K=1
            broadcast matmuls sit behind fresh scores in PE order, so the PE
            never stalls waiting on the DVE sums copies). For the last block,
            head 1's result goes to the partitions-0:63 staging tile (read
            against sb_wo2) instead of the slow cross-partition DMA."""
            isl = slice(ic * 512, (ic + 1) * 512)
            held = {}

            def step_sums():
                for hi in range(2):
                    s = normp.tile([65, 512], F16, tag="sums", name=f"sums{hi}")
                    nc.vector.tensor_copy(s[64:65, :], po[hi][64:65, :])
                    held[hi] = s

            def step_head(hi):
                if last and hi == 1:
                    # tail: the spare "o" slot is free; avoids serializing
                    # behind head 0's pb in the single "x" slot
                    pb = ps_o.tile([64, 512], F32, tag="o", name="pbz")
                else:
                    pb = ps_x.tile([64, 512], F32, tag="x", name=f"pb{hi}")
                nc.tensor.matmul(
                    pb[:],
                    lhsT=sb_one[64:65, :],
                    rhs=held[hi][64:65, :],
                    start=True,
                    stop=True,
                )
                rec = normp.tile([64, 512], F32, tag="rec", name=f"rec{hi}")
                nc.vector.reciprocal_approx_fast(rec[:], pb[:])
                if hi == 0:
                    nc.vector.tensor_mul(
                        sb_oT[0:64, p, isl], po[0][0:64, :], rec[:]
                    )
                elif last:
                    nc.vector.tensor_mul(sb_stage[:], po[1][0:64, :], rec[:])
                else:
                    tmpb = normp.tile([64, 512], BF16, tag="tmpb")
                    nc.vector.tensor_mul(tmpb[:], po[1][0:64, :], rec[:])
                    nc.sync.dma_start(sb_oT[64:128, p, isl], tmpb[:])

            return [step_sums, lambda: step_head(0), lambda: step_head(1)]

        # per-MM-granularity deferred projection chunks (run during p1 blocks)
        pending_proj = []
        proj_state = {"c2": None, "tile": None, "p": 0}

        def step_pending_proj():
            stt = proj_state
            if stt["c2"] is None:
                if not pending_proj:
                    return
                stt["c2"] = pending_proj.pop(0)
                stt["tile"] = ps_x.tile([128, 512], F32, tag="x", name="pf")
                stt["p"] = 0
            c2, p = stt["c2"], stt["p"]
            nc.tensor.matmul(
                stt["tile"][:],
                lhsT=sb_oT[:, p, c2 * 128 : (c2 + 1) * 128],
                rhs=sb_wo[:, p, :],
                start=(p == 0),
                stop=(p == 1),
            )
            stt["p"] += 1
            if stt["p"] == 2:
                fo = foutp.tile([128, 512], F32, tag="fo")
                nc.vector.tensor_copy(fo[:], stt["tile"][:])
                nc.sync.dma_start(out_d[c2 * 128 : (c2 + 1) * 128, :], fo[:])
                stt["c2"] = None

        def step_extras():
            """One deferred matmul: pending QK first, then projections."""
            if pending_qk or qk_state["chunk"] is not None:
                step_pending_qk()
            else:
                step_pending_proj()

        pending_norm = []
        blocks = [(p, ic) for p in range(2) for ic in range(4)]

        def emit_scores(p, ic, j):
            isl = slice(ic * 512, (ic + 1) * 512)
            jsl = slice(j * 128, (j + 1) * 128)
            st = ps_s.tile([128, 1024], F32, tag="s")
            nc.tensor.matmul(
                st[:, 0:512],
                lhsT=sb_kT[0:64, p, jsl],
                rhs=sb_qT[0:64, p, isl],
                start=True,
                stop=True,
            )
            nc.tensor.matmul(
                st[:, 512:1024],
                lhsT=sb_kT[64:128, p, jsl],
                rhs=sb_qT[64:128, p, isl],
                start=True,
                stop=True,
            )
            return st

        def emit_exp(st):
            pt = ptp.tile([128, 1024], BF16, tag="pt")
            nc.scalar.activation(pt[:], st[:], AF.Exp, scale=SCALE)
            return pt

        # Global score/exp stream runs TWO iterations ahead of the AV
        # stream: scores(k+2) issue before iteration k's AVs, so at block
        # boundaries the next block's first exps never queue behind the
        # last AVs (which themselves wait on the final exp of the block).
        # The 2-slot score pool self-paces the stream: scores(k+2) can't
        # run before EXP(k) has drained its slot.
        sched = [(p, ic, j) for (p, ic) in blocks for j in range(16)]
        pts = {}

        def emit_scores_exp(k):
            p, ic, j = sched[k]
            pts[k] = emit_exp(emit_scores(p, ic, j))

        emit_scores_exp(0)
        emit_scores_exp(1)
        po = None
        pending_burst = None

        def make_av(p_, po_):
            def emit_av(hi, jj, ptt):
                nc.tensor.matmul(
                    po_[hi][:],
                    lhsT=sb_v[:, jj, 2 * p_ + hi, 0:65],
                    rhs=ptt[:, hi * 512 : (hi + 1) * 512],
                    start=(jj == 0),
                    stop=(jj == 15),
                    skip_group_check=True,
                )

            return emit_av

        for k, (p, ic, j) in enumerate(sched):
            if j == 0:
                po = [
                    ps_o.tile([65, 512], F32, tag="o", name=f"po{hi}")
                    for hi in range(2)
                ]
                emit_av = make_av(p, po)
                if p == 1 and ic > 0:
                    # previous ic's projection slice; its oT inputs complete
                    # during this block's first two iterations (lazy norm)
                    pending_proj.extend(range(4 * (ic - 1), 4 * ic))
            if k + 2 < len(sched):
                emit_scores_exp(k + 2)
            if pending_burst is not None:
                # previous block's last three AVs + sums, emitted AFTER this
                # iteration's score pair so the next exps never queue behind
                # AVs that themselves wait on the previous block's last exp
                pending_burst()
                pending_burst = None
            if pending_norm:
                if j == 1:
                    pending_norm[0]()  # bcast+recip+mul head 0
                elif j == 2:
                    pending_norm[1]()  # ... head 1
                    pending_norm = []
            # extras: deferred matmuls keep PE fed; x-slot is needed
            # by the norm broadcasts at j=1,2 so extras wait till j>=3
            # (block 1 starts later still: it carries the densest deferred
            # stream and was PE-oversubscribed at j=3,4,5)
            if p == 0 and ic == 0:
                # K chunks 1-3 at 2 matmuls/iter over j=0..5; chunk c's copy
                # lands at j=2c-1, one iter before the 2-ahead score stream
                # for j=4c reads it
                if j <= 5:
                    c, half = 1 + j // 2, j % 2
                    if half == 0:
                        kq_tile = ps_o.tile([128, 512], F32, tag="o", name="pkh")
                    for kc in (0, 1) if half == 0 else (2, 3):
                        emit_qk_chunk_mm(sb_wk, 0, c, kc, kq_tile)
                    if half == 1:
                        nc.vector.tensor_copy(
                            sb_kT[:, 0, c * 512 : (c + 1) * 512], kq_tile[:]
                        )
                if 3 <= j < 15:
                    emit_v_chunk(j + 1)
            elif j >= (6 if (p == 0 and ic == 1) else 5):
                step_extras()

            # uniform AV lag (h0 by 1 iter, h1 by 2) keeps scores ahead
            # of the AV stream so ACT never waits at block boundaries
            if j >= 1:
                emit_av(0, j - 1, pts[k - 1])
            if j >= 2:
                emit_av(1, j - 2, pts[k - 2])
            if j == 15:

                def mk_burst(av_, p_, ic_, po_, k_, last_):
                    def burst():
                        nonlocal pending_norm
                        av_(0, 15, pts[k_])
                        av_(1, 14, pts[k_ - 1])
                        av_(1, 15, pts[k_])
                        pending_norm = make_norm_steps(p_, ic_, po_, last=last_)
                        pending_norm[0]()  # sums copies right behind the AVs
                        pending_norm = pending_norm[1:]

                    return burst

                pending_burst = mk_burst(
                    emit_av, p, ic, po, k, k == len(sched) - 1
                )
        pending_burst()  # last block's AV tail + sums

        # ---- tail: last normalize + remaining projection chunks ----
        # head 0 first: its chain is pure DVE and unblocks the oT top half
        pending_norm[0]()
        pending_norm[1]()
        while pending_proj or proj_state["c2"] is not None:
            step_pending_proj()
        # final four chunks, packed two per score-pool tile (those slots are
        # free the moment the last EXP consumed them): the pair-0 partials
        # depend only on long-finished oT pair-0 rows, so all four issue
        # immediately and overlap the norm chain above. The pair-1 matmuls
        # are split K=64: the top halves run off head 0's DVE-written oT
        # rows; the bottom halves contract the partitions-0:63 staging tile
        # against sb_wo2, so no cross-partition DMA gates the tail.
        # Groups interleave across tiles (skip_group_check).
        pfz = [ps_s.tile([128, 1024], F32, tag="s", name=f"pfz{n}") for n in range(2)]
        slots = [(c2, pfz[n // 2][:, (n % 2) * 512 : (n % 2 + 1) * 512])
                 for n, c2 in enumerate(range(12, 16))]
        for c2, pf in slots:
            nc.tensor.matmul(
                pf,
                lhsT=sb_oT[:, 0, c2 * 128 : (c2 + 1) * 128],
                rhs=sb_wo[:, 0, :],
                start=True,
                stop=False,
                skip_group_check=True,
            )
        for c2, pf in slots:
            nc.tensor.matmul(
                pf,
                lhsT=sb_oT[0:64, 1, c2 * 128 : (c2 + 1) * 128],
                rhs=sb_wo[0:64, 1, :],
                start=False,
                stop=False,
                skip_group_check=True,
            )
        for c2, pf in slots:
            nc.tensor.matmul(
                pf,
                lhsT=sb_stage[:, (c2 - 12) * 128 : (c2 - 11) * 128],
                rhs=sb_wo2[:],
                start=False,
                stop=True,
                skip_group_check=True,
            )
        for c2, pf in slots:
            fo = foutp.tile([128, 512], F32, tag="fo")
            nc.vector.tensor_copy(fo[:], pf)
            nc.sync.dma_start(out_d[c2 * 128 : (c2 + 1) * 128, :], fo[:])


def _build():
    nc = bacc.Bacc("TRN2", target_bir_lowering=False, debug=False, num_devices=N_CORES)
    xT = nc.dram_tensor("xT", [DM, S], BF16, kind="ExternalInput")
    wq = nc.dram_tensor("wq", [DM, DQ], BF16, kind="ExternalInput")
    wk = nc.dram_tensor("wk", [DM, DQ], BF16, kind="ExternalInput")
    wv = nc.dram_tensor("wv", [DM, DQ], BF16, kind="ExternalInput")
    wo = nc.dram_tensor("wo", [DQ, DM], BF16, kind="ExternalInput")
    out = nc.dram_tensor("out", [S, DM], F32, kind="ExternalOutput")
    with tile.TileContext(nc) as tc:
        _kernel_body(tc, xT.ap(), wq.ap(), wk.ap(), wv.ap(), wo.ap(), out.ap())
    nc.compile()
    return nc


def get_nc():
    global _CACHED_NC
    if _CACHED_NC is None:
        _CACHED_NC = _build()
    return _CACHED_NC


def _in_maps(hidden_states, Wq, Wk, Wv, Wo):
    bf = ml_dtypes.bfloat16
    maps = []
    for c in range(N_CORES):
        b, g = c // 2, c % 2
        cols = slice(g * DQ, (g + 1) * DQ)
        maps.append(
            {
                "xT": np.ascontiguousarray(hidden_states[b].T).astype(bf),
                "wq": np.ascontiguousarray(Wq[:, cols]).astype(bf),
                "wk": np.ascontiguousarray(Wk[:, cols]).astype(bf),
                "wv": np.ascontiguousarray(Wv[:, cols]).astype(bf),
                "wo": np.ascontiguousarray(Wo[cols, :]).astype(bf),
            }
        )
    return maps


def _ensure_profile_support():
    """Best-effort: register the axon NTFF profiling hook + defang the
    bucket upload (zero-egress container). Without this, trace=True dies
    on a missing ``antenv.axon_hooks`` module in this image."""
    import types

    try:
        import antenv

        if "antenv.axon_hooks" not in sys.modules:
            mod = types.ModuleType("antenv.axon_hooks")
            _h = {"hook": None}
            mod.set_axon_ntff_profile_hook = lambda h: _h.__setitem__("hook", h)
            mod.get_axon_ntff_profile_hook = lambda: _h["hook"]
            sys.modules["antenv.axon_hooks"] = mod
            antenv.axon_hooks = mod
        import antenv.axon_hooks as ah

        if ah.get_axon_ntff_profile_hook() is None:
            if "/root/.axon_site" not in sys.path:
                sys.path.append("/root/.axon_site")
            from trn_agent_boot.trn_boot import _ntff_profile_via_ctypes

            hook = _ntff_profile_via_ctypes("/opt/axon/libaxon_pjrt.so")
            if hook is not None:
                ah.set_axon_ntff_profile_hook(hook)
    except Exception:
        pass
    try:
        import concourse.bass_utils as bu

        bu.upload_artifacts = lambda tmpdir: tmpdir
    except Exception:
        pass


def kernel(hidden_states, Wq, Wk, Wv, Wo):
    global LAST_EXEC_TIME_NS, LAST_RESULT
    hidden_states = np.asarray(hidden_states, dtype=np.float32)
    Wq, Wk, Wv, Wo = (np.asarray(w, dtype=np.float32) for w in (Wq, Wk, Wv, Wo))

    trace = bool(os.environ.get("BASS_TRACE"))
    if trace:
        _ensure_profile_support()
    nc = get_nc()
    maps = _in_maps(hidden_states, Wq, Wk, Wv, Wo)
    res = run_bass_kernel_spmd(
        nc,
        maps,
        core_ids=list(range(N_CORES)),
        trace=trace,
        tmpdir=os.environ.get("BASS_TRACE_DIR") or None,
    )
    LAST_RESULT = res
    LAST_EXEC_TIME_NS = res.exec_time_ns

    out = np.empty((B, S, DM), dtype=np.float32)
    for b in range(B):
        out[b] = res.results[2 * b]["out"] + res.results[2 * b + 1]["out"]
    return out


if __name__ == "__main__":
    rng = np.random.default_rng(0)
    hs = rng.standard_normal((B, S, DM), dtype=np.float32)
    ws = [
        (rng.standard_normal((DM, DM), dtype=np.float32) / np.sqrt(DM))
        for _ in range(4)
    ]
    o = kernel(hs, *ws)
    print("out", o.shape, o.dtype, float(np.abs(o).mean()))
    print("exec_time_ns", LAST_EXEC_TIME_NS)
